# revision 30
# baseline (speedup 1.0000x reference)
"""Trainium2 Bass kernel for nn_CS_MAMBA (pool -> mamba -> channel-attention -> FFN).

Data-parallel over batch: 64 batch items sharded 8-per-core across 8 NeuronCores;
all weights replicated. Per core the 8 batch items are processed in 4 groups of
GB=2: the pool+mamba front-end of group g+1 overlaps the FFN matmuls of group g.
Column convention throughout the front-end: col = j*L + l (batch-in-group outer,
sequence INNER) so the selective scan is a single tensor_tensor_scan.
FFN matmuls pair the two batches of a group into 576-col moving operands.
"""

import numpy as np
import ml_dtypes

# ---------------------------------------------------------------- constants
B_FULL = 64
N_CORES = 8
BL = B_FULL // N_CORES          # 8 batch items per core
GB = 4                          # batch-group size
NG = BL // GB                   # 4 groups
C = 2048
NCT = C // 128                  # 16 channel tiles
H, W = 24, 12
HW = H * W                      # 288
POOL_W = 48                     # elements summed per patch (4 rows x 12 cols)
L = 12                          # interleaved sequence length
COLSG = L * GB                  # 24 group-local columns, col = j*L + l
DI = 256                        # d_inner
DIT = DI // 128                 # 2 d_inner tiles
DS = 16                         # d_state
DTR = 16                        # dt_rank
EPS = 1e-5

# packed per-partition small constants: name -> number of [128, n] columns
SMALLS = [
    ("wx", DIT * 48),       # Wx.T as [128, 2, 48]
    ("cw", DIT * 3),        # conv w as [128, 2, 3]
    ("ncb", DIT),           # -conv_b
    ("bdt", DIT),
    ("dssm", DIT),
    ("A3", DIT * DS),       # -exp(A_log) as [128, 2, 16]
    ("nwg", DIT),           # -(W_in @ ln1_g) as [128, 2]
    ("wb", DIT),            # W_in @ ln1_b as [128, 2]
    ("ln2g", NCT), ("ln2b", NCT),
    ("absx", 1), ("abnb", 1),
    ("fvs", NCT), ("fvb", NCT), ("fis", NCT), ("fib", NCT),
]
SM_OFF = {}
_off = 0
for _n, _w in SMALLS:
    SM_OFF[_n] = (_off, _off + _w)
    _off += _w
SM_COLS = _off

_CACHE = {}


def _build(nc_mod, tile_mod, mybir, masks, repeat=1, parts="all"):
    """Emit the bass program. Returns the compiled Bass object."""
    F32 = mybir.dt.float32
    BF16 = mybir.dt.bfloat16
    AF = mybir.ActivationFunctionType
    ALU = mybir.AluOpType
    AX = mybir.AxisListType

    nc = nc_mod.Bacc("TRN2", target_bir_lowering=False, debug=False)

    # ---------------- dram tensors (names = in_map keys)
    d_vis = nc.dram_tensor("vis", [BL, C, HW], F32, kind="ExternalInput")
    d_inf = nc.dram_tensor("inf", [BL, C, HW], F32, kind="ExternalInput")
    d_sm = nc.dram_tensor("smalls", [128, SM_COLS], F32, kind="ExternalInput")
    d_winT = nc.dram_tensor("w_inT", [128, NCT, DI], BF16, kind="ExternalInput")
    d_wdtT = nc.dram_tensor("wdtT", [DTR, DI], F32, kind="ExternalInput")
    d_woutT = nc.dram_tensor("w_outT", [128, DIT, C], BF16, kind="ExternalInput")
    d_aw1T = nc.dram_tensor("aw1T", [128, NCT, 128], BF16, kind="ExternalInput")
    d_aw2T = nc.dram_tensor("aw2T", [128, C], BF16, kind="ExternalInput")
    d_wvT = nc.dram_tensor("wvT", [128, NCT, C], BF16, kind="ExternalInput")
    d_wiT = nc.dram_tensor("wiT", [128, NCT, C], BF16, kind="ExternalInput")

    d_out_vis = nc.dram_tensor("out_vis", [BL, C, HW], F32, kind="ExternalOutput")
    d_out_inf = nc.dram_tensor("out_inf", [BL, C, HW], F32, kind="ExternalOutput")

    fm_d = [d_vis, d_inf]
    out_d = [d_out_vis, d_out_inf]

    with tile_mod.TileContext(nc) as tc:
        with (
            tc.tile_pool(name="consts", bufs=1) as consts,
            tc.tile_pool(name="wpool", bufs=1) as wpool,
            tc.tile_pool(name="stream", bufs=3) as stream,
            tc.tile_pool(name="axp", bufs=4) as axpool,
            tc.tile_pool(name="outp", bufs=2) as outp,
            tc.tile_pool(name="vip", bufs=1) as vip,
            tc.tile_pool(name="mam", bufs=1) as mam,
            tc.tile_pool(name="psA", bufs=4, space="PSUM") as psA,
            tc.tile_pool(name="psB", bufs=1, space="PSUM") as psB,
            tc.tile_pool(name="psC", bufs=2, space="PSUM") as psC,
        ):
            # ---------------- constants / weights to SBUF
            ident = consts.tile([128, 128], F32)
            masks.make_identity(nc, ident)
            ones_col = consts.tile([128, 1], F32)
            nc.vector.memset(ones_col, 1.0)
            ones_col_bf = consts.tile([128, 1], BF16)
            nc.vector.memset(ones_col_bf, 1.0)
            ones_row = consts.tile([1, 128], F32)
            nc.vector.memset(ones_row, 1.0)
            ones_row_bf = consts.tile([1, 128], BF16)
            nc.vector.memset(ones_row_bf, 1.0)
            eps1 = consts.tile([1, 1], F32)
            nc.vector.memset(eps1, EPS * POOL_W * POOL_W)   # LN1 stats on 48x sums
            eps2 = consts.tile([1, 1], F32)
            nc.vector.memset(eps2, EPS)

            sm = consts.tile([128, SM_COLS], F32)
            nc.gpsimd.dma_start(out=sm, in_=d_sm[:, :])

            def smv(name, i3=None):
                a, b = SM_OFF[name]
                v = sm[:, a:b]
                if i3 is not None:
                    v = v.rearrange("p (i k) -> p i k", i=i3)
                return v

            wxT = smv("wx", DIT)
            cw3 = smv("cw", DIT)
            ncb = smv("ncb")
            bdt2 = smv("bdt")
            dssm2 = smv("dssm")
            A3 = smv("A3", DIT)
            nwg = smv("nwg")
            wb2 = smv("wb")
            ln2g, ln2b = smv("ln2g"), smv("ln2b")
            absx, abnb = smv("absx"), smv("abnb")
            fvs, fvb = smv("fvs"), smv("fvb")
            fis, fib = smv("fis"), smv("fib")

            winT = consts.tile([128, NCT, DI], BF16)
            nc.gpsimd.dma_start(out=winT, in_=d_winT[:, :, :])
            wdtT = consts.tile([DTR, DI], F32)
            nc.gpsimd.dma_start(out=wdtT, in_=d_wdtT[:, :])
            woutT = consts.tile([128, DIT, C], BF16)
            aw1T = consts.tile([128, NCT, 128], BF16)
            aw2T = consts.tile([128, C], BF16)

            def load_consts2(eng):
                eng.dma_start(out=woutT, in_=d_woutT[:, :, :])
                eng.dma_start(out=aw1T, in_=d_aw1T[:, :, :])
                eng.dma_start(out=aw2T, in_=d_aw2T[:, :])

            import contextlib
            rep_ctx = tc.For_i(0, repeat, 1) if repeat > 1 else contextlib.nullcontext()
            with rep_ctx:
                # attention per group: [128, NCT, 2(s), GB(j)]
                att_g = [
                    consts.tile([128, NCT, 2, GB], F32, name=f"att{g}")
                    for g in range(NG)
                ]

                # ============================================================
                # Front-end. col = j*L + l, l = 2*pp + s.
                # ============================================================
                def pool_group(g):
                    Vi = vip.tile([128, NCT, COLSG], F32, tag="vi", bufs=NG,
                                  name=f"Vi{g}")
                    for cq in range(2):
                        for j in range(GB):
                            b = g * GB + j
                            for s in range(2):
                                ft = stream.tile(
                                    [128, 8, HW], F32, tag="fm",
                                    name=f"pfm{g}_{s}_{j}_{cq}",
                                )
                                nc.sync.dma_start(
                                    out=ft,
                                    in_=fm_d[s][b, cq * 1024:(cq + 1) * 1024, :]
                                    .rearrange("(a p f) w -> p a f w", a=2, f=4),
                                )
                                # windowed sums -> Vi[:, kc, j*L + 2*pp + s]
                                nc.vector.reduce_sum(
                                    out=Vi[:, cq * 8:(cq + 1) * 8, :].rearrange(
                                        "p c (j pp two) -> p c j pp two",
                                        j=GB, two=2,
                                    )[:, :, j, :, s],
                                    in_=ft.rearrange(
                                        "p c (pp w) -> p c pp w", w=POOL_W
                                    ),
                                    axis=AX.X,
                                )
                    return Vi

                def mamba_group(g, Vi):
                    # ---- LN1 stats on Vi (48x scale; eps1 compensates)
                    sq = mam.tile([128, NCT, COLSG], BF16, tag="lnsq", name="sq")
                    nc.scalar.activation(out=sq[:, :, :], in_=Vi[:, :, :],
                                         func=AF.Square)
                    vib = mam.tile([128, NCT, COLSG], BF16, tag="vib", name="vib")
                    nc.scalar.activation(out=vib[:, :, :], in_=Vi[:, :, :],
                                         func=AF.Copy)
                    s1p = psC.tile([128, COLSG], F32, tag="psS", name=f"s1p{g}")
                    s2p = psC.tile([128, COLSG], F32, tag="psS", name=f"s2p{g}")
                    for ci in range(NCT):
                        nc.tensor.matmul(
                            s1p[0:1, :], ones_col, Vi[:, ci, :],
                            start=(ci == 0), stop=(ci == NCT - 1),
                        )
                        nc.tensor.matmul(
                            s2p[0:1, :], ones_col_bf, sq[:, ci, :],
                            start=(ci == 0), stop=(ci == NCT - 1),
                        )

                    def ln_stats(s1, s2, epsv, tag):
                        m_sb = mam.tile([1, COLSG], F32, tag=f"m{tag}", name="m")
                        nc.vector.tensor_scalar_mul(m_sb, s1[0:1, :], 1.0 / C)
                        v_sb = mam.tile([1, COLSG], F32, tag=f"v{tag}", name="v")
                        nc.vector.tensor_scalar_mul(v_sb, s2[0:1, :], 1.0 / C)
                        msq = mam.tile([1, COLSG], F32, tag=f"msq{tag}", name="msq")
                        nc.vector.tensor_mul(msq, m_sb, m_sb)
                        nc.vector.tensor_sub(v_sb, v_sb, msq)
                        r_sb = mam.tile([1, COLSG], F32, tag=f"r{tag}", name="r")
                        nc.scalar.activation(out=r_sb, in_=v_sb, func=AF.Ln,
                                             bias=epsv)
                        nc.scalar.activation(out=r_sb, in_=r_sb, func=AF.Exp,
                                             scale=-0.5)
                        mr_sb = mam.tile([1, COLSG], F32, tag=f"mr{tag}", name="mr")
                        nc.vector.tensor_mul(mr_sb, m_sb, r_sb)
                        rbm = psC.tile([128, 2, COLSG], F32, tag="psS",
                                       name=f"rbm{tag}{g}")
                        nc.tensor.matmul(rbm[:, 0, :], ones_row, r_sb,
                                         start=True, stop=True)
                        nc.tensor.matmul(rbm[:, 1, :], ones_row, mr_sb,
                                         start=True, stop=True)
                        return rbm[:, 0, :], rbm[:, 1, :]

                    rb1, mrb1 = ln_stats(s1p, s2p, eps1, "a")

                    # ---- P = W_in' @ Vi ; x1 = rb*P + mrb*nwg + wb
                    Pp = psC.tile([128, DIT, COLSG], F32, tag="psS", name=f"Pp{g}")
                    for i in range(DIT):
                        for ci in range(NCT):
                            nc.tensor.matmul(
                                Pp[:, i, :], winT[:, ci, i * 128:(i + 1) * 128],
                                vib[:, ci, :],
                                start=(ci == 0), stop=(ci == NCT - 1),
                            )
                    x1 = mam.tile([128, DIT, COLSG], F32, tag="x1", name=f"x1{g}")
                    rb1_bc = rb1.unsqueeze(1).broadcast_to([128, DIT, COLSG])
                    nc.vector.tensor_copy(out=x1[:, :, :], in_=Pp[:, :, :])
                    nc.vector.tensor_tensor(out=x1[:, :, :], in0=x1[:, :, :],
                                            in1=rb1_bc, op=ALU.mult)
                    for i in range(DIT):
                        nc.vector.scalar_tensor_tensor(
                            out=x1[:, i, :], in0=mrb1[:, :], scalar=nwg[:, i:i + 1],
                            in1=x1[:, i, :], op0=ALU.mult, op1=ALU.add,
                        )
                        nc.vector.tensor_scalar_add(
                            out=x1[:, i, :], in0=x1[:, i, :],
                            scalar1=wb2[:, i:i + 1],
                        )

                    # ---- depthwise conv (along l, guarded per j) + silu
                    cv = mam.tile([128, DIT, COLSG], F32, tag="cv", name=f"cv{g}")
                    e_t = mam.tile([128, DIT, COLSG], F32, tag="e_t", name=f"e_t{g}")
                    xact = mam.tile([128, DIT, COLSG], F32, tag="xact",
                                    name=f"xact{g}")
                    for i in range(DIT):
                        nc.vector.tensor_scalar_mul(
                            out=cv[:, i, :], in0=x1[:, i, :], scalar1=cw3[:, i, 1:2]
                        )
                        cvj = cv[:, i, :].rearrange("p (j l) -> p j l", j=GB)
                        x1j = x1[:, i, :].rearrange("p (j l) -> p j l", j=GB)
                        nc.vector.scalar_tensor_tensor(
                            out=cvj[:, :, 1:L], in0=x1j[:, :, 0:L - 1],
                            scalar=cw3[:, i, 0:1], in1=cvj[:, :, 1:L],
                            op0=ALU.mult, op1=ALU.add,
                        )
                        nc.vector.scalar_tensor_tensor(
                            out=cvj[:, :, 0:L - 1], in0=x1j[:, :, 1:L],
                            scalar=cw3[:, i, 2:3], in1=cvj[:, :, 0:L - 1],
                            op0=ALU.mult, op1=ALU.add,
                        )
                        # silu(cv + cb) = (cv+cb)/(1+exp(-(cv+cb))); ncb = -conv_b
                        nc.scalar.activation(
                            out=e_t[:, i, :], in_=cv[:, i, :], func=AF.Exp,
                            scale=-1.0, bias=ncb[:, i:i + 1],
                        )
                    nc.vector.tensor_scalar_add(out=e_t[:, :, :], in0=e_t[:, :, :],
                                                scalar1=1.0)
                    nc.vector.reciprocal(out=e_t[:, :, :], in_=e_t[:, :, :])
                    for i in range(DIT):
                        nc.vector.scalar_tensor_tensor(
                            out=xact[:, i, :], in0=cv[:, i, :],
                            scalar=ncb[:, i:i + 1],
                            in1=e_t[:, i, :], op0=ALU.subtract, op1=ALU.mult,
                        )

                    # ---- dbc = x @ Wx.T -> [col, 48]; transpose -> [48, col]
                    dbcp = psC.tile([128, 48], F32, tag="psS", name=f"dbcp{g}")
                    for i in range(DIT):
                        nc.tensor.matmul(
                            dbcp[0:COLSG, :], xact[:, i, :], wxT[:, i, :],
                            start=(i == 0), stop=(i == DIT - 1),
                        )
                    dbc_sb = mam.tile([COLSG, 48], F32, tag="dbc", name=f"dbc{g}")
                    nc.vector.tensor_copy(out=dbc_sb, in_=dbcp[0:COLSG, :])
                    dtp = psC.tile([128, COLSG], F32, tag="psS", name=f"dtp{g}")
                    nc.tensor.transpose(
                        dtp[0:48, :], dbc_sb[:, :], ident[0:COLSG, 0:COLSG]
                    )
                    dbcT = mam.tile([48, COLSG], F32, tag="dbcT", name=f"dbcT{g}")
                    nc.vector.tensor_copy(out=dbcT, in_=dtp[0:48, :])

                    # ---- delta = softplus(dT @ Wdt.T + bdt) -> [256, col]
                    delta = mam.tile([128, DIT, COLSG], F32, tag="delta",
                                     name=f"delta{g}")
                    for i in range(DIT):
                        dp = psC.tile([128, COLSG], F32, tag="psS", name=f"dp{g}_{i}")
                        nc.tensor.matmul(
                            dp, wdtT[:, i * 128:(i + 1) * 128], dbcT[0:DTR, :],
                            start=True, stop=True,
                        )
                        nc.scalar.activation(
                            out=delta[:, i, :], in_=dp, func=AF.Exp,
                            bias=bdt2[:, i:i + 1],
                        )
                    nc.scalar.activation(out=delta[:, :, :], in_=delta[:, :, :],
                                         func=AF.Ln, bias=1.0)

                    # ---- Bp/Cp row-broadcast: flat [1, (n j l)] -> [128, (n j l)]
                    bcT_bf = mam.tile([48, COLSG], BF16, tag="bcTb",
                                      name=f"bcTb{g}")
                    nc.vector.tensor_copy(out=bcT_bf, in_=dtp[0:48, :])
                    bc_flat = mam.tile([1, 2 * DS * COLSG], BF16, tag="flat",
                                       name=f"bcf{g}")
                    nc.gpsimd.dma_start(
                        out=bc_flat.rearrange("o (n c) -> o n c", n=2 * DS),
                        in_=bcT_bf[DTR:DTR + 2 * DS, :],
                    )
                    NBC = DS * COLSG
                    bpb = psB.tile([128, NBC], F32, tag="big", bufs=1, name=f"bpb{g}")
                    cpb_ps = psB.tile([128, NBC], F32, tag="big", bufs=1,
                                      name=f"cpp{g}")
                    for k in range((NBC + 511) // 512):
                        sl = slice(512 * k, min(NBC, 512 * (k + 1)))
                        nc.tensor.matmul(bpb[:, sl], ones_row_bf, bc_flat[:, sl],
                                         start=True, stop=True)
                        nc.tensor.matmul(
                            cpb_ps[:, sl], ones_row_bf,
                            bc_flat[:, NBC + 512 * k:NBC + sl.stop],
                            start=True, stop=True)
                    cpb = mam.tile([128, DS * COLSG], BF16, tag="cpbs", name=f"cpb{g}")
                    nc.vector.tensor_copy(out=cpb, in_=cpb_ps)

                    # ---- dA = exp(delta x A), memory order [128, (i n j l)]
                    dA = mam.tile([128, DIT, DS * COLSG], BF16, tag="dA",
                                  name=f"dA{g}")
                    for i in range(DIT):
                        nc.vector.tensor_tensor(
                            out=dA[:, i, :].rearrange("p (n c) -> p n c", n=DS),
                            in0=delta[:, i, :].unsqueeze(1)
                            .broadcast_to([128, DS, COLSG]),
                            in1=A3[:, i, :].unsqueeze(2)
                            .broadcast_to([128, DS, COLSG]),
                            op=ALU.mult,
                        )
                    nc.scalar.activation(out=dA[:, :, :], in_=dA[:, :, :],
                                         func=AF.Exp)
                    # zero dA at l==0 of every (i,n,j) block -> scan resets there
                    nc.vector.tensor_scalar_mul(
                        out=dA.rearrange("p i (b l) -> p (i b) l", l=L)[:, :, 0:1],
                        in0=dA.rearrange("p i (b l) -> p (i b) l", l=L)[:, :, 0:1],
                        scalar1=0.0,
                    )

                    # ---- dBu = (delta*x) x Bp
                    du = mam.tile([128, DIT, COLSG], F32, tag="du", name=f"du{g}")
                    nc.vector.tensor_mul(du[:, :, :], delta[:, :, :], xact[:, :, :])
                    dBu = mam.tile([128, DIT, DS * COLSG], BF16, tag="dBu",
                                   name=f"dBu{g}")
                    for i in range(DIT):
                        nc.vector.tensor_tensor(
                            out=dBu[:, i, :].rearrange("p (n c) -> p n c", n=DS),
                            in0=du[:, i, :].unsqueeze(1)
                            .broadcast_to([128, DS, COLSG]),
                            in1=bpb.rearrange("p (n c) -> p n c", n=DS),
                            op=ALU.mult,
                        )

                    # ---- selective scan: one pass over (i n j l), l innermost
                    Hs = mam.tile([128, DIT * DS * COLSG], BF16, tag="Hs",
                                  name=f"Hs{g}")
                    nc.vector.tensor_tensor_scan(
                        out=Hs[:, :],
                        data0=dA.rearrange("p i c -> p (i c)"),
                        data1=dBu.rearrange("p i c -> p (i c)"),
                        initial=0.0, op0=ALU.mult, op1=ALU.add,
                    )

                    # ---- y = sum_n H * Cp  (+ x * D_ssm)
                    tt = mam.tile([128, DIT, DS * COLSG], BF16, tag="dA",
                                  name=f"yt{g}")
                    nc.vector.tensor_tensor(
                        out=tt[:, :, :],
                        in0=Hs.rearrange("p (i c) -> p i c", i=DIT),
                        in1=cpb.unsqueeze(1).broadcast_to([128, DIT, DS * COLSG]),
                        op=ALU.mult,
                    )
                    y2 = mam.tile([128, DIT, COLSG], F32, tag="y2", name=f"y2{g}")
                    nc.vector.reduce_sum(
                        out=y2[:, :, :],
                        in_=tt.rearrange("p i (n c) -> p i c n", n=DS),
                        axis=AX.X,
                    )
                    for i in range(DIT):
                        nc.vector.scalar_tensor_tensor(
                            out=y2[:, i, :], in0=xact[:, i, :],
                            scalar=dssm2[:, i:i + 1],
                            in1=y2[:, i, :], op0=ALU.mult, op1=ALU.add,
                        )
                    y2b = mam.tile([128, DIT, COLSG], BF16, tag="y2b", name=f"y2b{g}")
                    nc.scalar.activation(out=y2b[:, :, :], in_=y2[:, :, :],
                                         func=AF.Copy)

                    # ---- vi2 = y2 @ W_out.T + Vi/48
                    vi2 = vip.tile([128, NCT, COLSG], F32, tag="v24", bufs=2,
                                   name=f"vi2{g}")
                    vi2p = psB.tile([128, NCT, 64], F32, tag="big", bufs=1,
                                    name=f"vi2p{g}")
                    for mc in range(NCT):
                        for i in range(DIT):
                            nc.tensor.matmul(
                                vi2p[:, mc, 0:COLSG],
                                woutT[:, i, mc * 128:(mc + 1) * 128], y2b[:, i, :],
                                start=(i == 0), stop=(i == DIT - 1),
                            )
                    nc.vector.scalar_tensor_tensor(
                        out=vi2[:, :, :], in0=Vi[:, :, :], scalar=1.0 / POOL_W,
                        in1=vi2p[:, :, 0:COLSG], op0=ALU.mult, op1=ALU.add,
                    )

                    # ---- LN2 (generic g/b)
                    sq2 = mam.tile([128, NCT, COLSG], BF16, tag="lnsq", name="sq2")
                    nc.scalar.activation(out=sq2[:, :, :], in_=vi2[:, :, :],
                                         func=AF.Square)
                    t1p = psC.tile([128, COLSG], F32, tag="psS", name=f"t1p{g}")
                    t2p = psC.tile([128, COLSG], F32, tag="psS", name=f"t2p{g}")
                    for ci in range(NCT):
                        nc.tensor.matmul(
                            t1p[0:1, :], ones_col, vi2[:, ci, :],
                            start=(ci == 0), stop=(ci == NCT - 1),
                        )
                        nc.tensor.matmul(
                            t2p[0:1, :], ones_col_bf, sq2[:, ci, :],
                            start=(ci == 0), stop=(ci == NCT - 1),
                        )
                    rb2, mrb2 = ln_stats(t1p, t2p, eps2, "b")
                    nrm = vip.tile([128, NCT, COLSG], F32, tag="v24", bufs=2,
                                   name=f"nrm{g}")
                    rb2_bc = rb2.unsqueeze(1).broadcast_to([128, NCT, COLSG])
                    mrb2_bc = mrb2.unsqueeze(1).broadcast_to([128, NCT, COLSG])
                    g_bc = ln2g.unsqueeze(2).broadcast_to([128, NCT, COLSG])
                    b_bc = ln2b.unsqueeze(2).broadcast_to([128, NCT, COLSG])
                    nc.vector.tensor_tensor(out=nrm[:, :, :], in0=vi2[:, :, :],
                                            in1=rb2_bc, op=ALU.mult)
                    nc.vector.tensor_tensor(out=nrm[:, :, :], in0=nrm[:, :, :],
                                            in1=mrb2_bc, op=ALU.subtract)
                    nc.vector.tensor_tensor(out=nrm[:, :, :], in0=nrm[:, :, :],
                                            in1=g_bc, op=ALU.mult)
                    nc.vector.tensor_tensor(out=nrm[:, :, :], in0=nrm[:, :, :],
                                            in1=b_bc, op=ALU.add)

                    # ---- channel attention, both streams + mean/max merged.
                    # mvx cols per c: (src(2), s(2), j(2)); col = j*L + 2*pp + s
                    mvx = mam.tile([128, NCT, 4 * GB], F32, tag="mvx",
                                   name=f"mvx{g}")
                    mvx5 = mvx.rearrange("p c (x s j) -> p c x s j", x=2, s=2)
                    nv = nrm.rearrange("p c (j pp two) -> p c two j pp", j=GB,
                                       two=2)
                    for s in range(2):
                        nc.vector.reduce_sum(out=mvx5[:, :, 0, s, :],
                                             in_=nv[:, :, s, :, :], axis=AX.X)
                        nc.vector.reduce_max(out=mvx5[:, :, 1, s, :],
                                             in_=nv[:, :, s, :, :], axis=AX.X)
                    # mean path: sum/6
                    nc.vector.tensor_scalar_mul(
                        out=mvx.rearrange("p c (x sj) -> p c x sj", x=2)[:, :, 0, :],
                        in0=mvx.rearrange("p c (x sj) -> p c x sj", x=2)[:, :, 0, :],
                        scalar1=1.0 / (L // 2),
                    )
                    mvxb = mam.tile([128, NCT, 4 * GB], BF16, tag="mvxb",
                                    name=f"mvxb{g}")
                    nc.scalar.activation(out=mvxb[:, :, :],
                                         in_=mvx[:, :, :], func=AF.Copy)
                    hp = psC.tile([128, COLSG], F32, tag="psS", name=f"hp{g}")
                    for ci in range(NCT):
                        nc.tensor.matmul(
                            hp[:, 0:4 * GB], aw1T[:, ci, :],
                            mvxb[:, ci, :],
                            start=(ci == 0), stop=(ci == NCT - 1),
                        )
                    h1 = mam.tile([128, 2, 2 * GB], BF16, tag="h1", name=f"h1{g}")
                    nc.scalar.activation(
                        out=h1[:, :, :], in_=hp[:, 0:4 * GB], func=AF.Relu,
                        scale=absx, bias=abnb,
                    )
                    h1s = mam.tile([128, 2 * GB], BF16, tag="h1s", name=f"h1s{g}")
                    nc.vector.tensor_tensor(out=h1s, in0=h1[:, 0, :],
                                            in1=h1[:, 1, :], op=ALU.add)
                    att = att_g[g]
                    apb = psB.tile([128, NCT, 2 * GB], F32, tag="big", bufs=1,
                                   name=f"apb{g}")
                    for mc in range(NCT):
                        nc.tensor.matmul(
                            apb[:, mc, :], aw2T[:, mc * 128:(mc + 1) * 128], h1s,
                            start=True, stop=True,
                        )
                    nc.scalar.activation(
                        out=att[:, :, :, :], in_=apb[:, :, :], func=AF.Exp,
                        scale=-1.0,
                    )
                    nc.vector.tensor_scalar_add(out=att[:, :, :, :],
                                                in0=att[:, :, :, :], scalar1=1.0)
                    nc.vector.reciprocal(out=att[:, :, :, :], in_=att[:, :, :, :])

                # ============================================================
                # FFN: per (group, stream) pair of batches, 576-col matmuls.
                # ============================================================
                scl_s = [fvs, fis]
                scl_b = [fvb, fib]
                w_dram = [d_wvT, d_wiT]
                wts = {}

                def load_w(s, eng=None):
                    eng = eng or nc.gpsimd
                    wt = [
                        wpool.tile([128, C], BF16, tag="w", bufs=17,
                                   name=f"w{s}_{kc}")
                        for kc in range(NCT)
                    ]
                    for kc in range(NCT):
                        eng.dma_start(out=wt[kc], in_=w_dram[s][:, kc, :])
                    wts[s] = wt

                def ffn_pair(g, s, skip_io=False):
                    wt = wts[s]
                    att = att_g[g]
                    for j in range(GB):
                        b = g * GB + j
                        axt = [
                            axpool.tile([128, 8, HW], BF16, tag="ax", bufs=4,
                                        name=f"ax{s}_{g}_{j}_{cq}")
                            for cq in range(2)
                        ]
                        if skip_io:
                            for cq in range(2):
                                nc.gpsimd.memset(axt[cq], 0.01)
                        else:
                            for cq in range(2):
                                ft = stream.tile(
                                    [128, 8, HW], F32, tag="fm",
                                    name=f"ffm{s}_{g}_{j}_{cq}",
                                )
                                nc.sync.dma_start(
                                    out=ft,
                                    in_=fm_d[s][b, cq * 1024:(cq + 1) * 1024, :]
                                    .rearrange("(a p f) w -> p a f w", a=2, f=4),
                                )
                                if cq == 0:
                                    nc.vector.tensor_tensor(
                                        out=axt[cq][:, :, :],
                                        in0=ft[:, :, :],
                                        in1=att[:, 0:8, s, j]
                                        .unsqueeze(2).broadcast_to([128, 8, HW]),
                                        op=ALU.mult,
                                    )
                                else:
                                    for kl in range(8):
                                        nc.scalar.activation(
                                            out=axt[cq][:, kl, :],
                                            in_=ft[:, kl, :], func=AF.Copy,
                                            scale=att[:, 8 + kl, s, j:j + 1],
                                        )
                        for mq in range(NCT // 4):
                            ot = outp.tile(
                                [128, 4, HW], F32, tag="ot",
                                name=f"ot{s}_{g}_{j}_{mq}"
                            )
                            for mi in range(4):
                                mc = mq * 4 + mi
                                pp = psA.tile([128, HW], F32, tag="pp", bufs=4,
                                              name=f"pp{s}_{g}_{j}_{mc}")
                                for kc in range(NCT):
                                    nc.tensor.matmul(
                                        pp, wt[kc][:, mc * 128:(mc + 1) * 128],
                                        axt[kc // 8][:, kc % 8, :],
                                        start=(kc == 0), stop=(kc == NCT - 1),
                                    )
                                nc.scalar.activation(
                                    out=ot[:, mi, :], in_=pp, func=AF.Relu,
                                    scale=scl_s[s][:, mc:mc + 1],
                                    bias=scl_b[s][:, mc:mc + 1],
                                )
                            if not skip_io:
                                nc.scalar.dma_start(
                                    out=out_d[s][b, mq * 512:(mq + 1) * 512, :]
                                    .rearrange("(p four) w -> p four w", four=4),
                                    in_=ot,
                                )

                # ---- emission: software-pipelined fronts and FFN stages
                if parts != "all":
                    load_consts2(nc.gpsimd)
                if parts == "front":
                    for g in range(NG):
                        Vi_g = pool_group(g)
                        mamba_group(g, Vi_g)
                elif parts == "pool":
                    for g in range(NG):
                        pool_group(g)
                elif parts == "mamba":
                    for g in range(NG):
                        Vi_g = vip.tile([128, NCT, COLSG], F32, tag="vi", bufs=NG,
                                        name=f"Vi{g}")
                        nc.vector.memset(Vi_g, 0.5)
                        mamba_group(g, Vi_g)
                elif parts in ("ffn", "mm"):
                    for gg in range(NG):
                        nc.vector.memset(att_g[gg], 1.0)
                    load_w(0)
                    for g in range(NG):
                        ffn_pair(g, 0, skip_io=(parts == "mm"))
                    load_w(1)
                    for g in range(NG):
                        ffn_pair(g, 1, skip_io=(parts == "mm"))
                elif parts == "io":
                    iot = consts.tile([128, 8, HW], F32)
                    nc.vector.memset(iot, 0.25)
                    for s in range(2):
                        for b in range(BL):
                            for cq in range(2):
                                ft = stream.tile([128, 8, HW], F32, tag="fm",
                                                 name=f"ioi{s}_{b}_{cq}")
                                nc.sync.dma_start(
                                    out=ft,
                                    in_=fm_d[s][b, cq * 1024:(cq + 1) * 1024, :]
                                    .rearrange("(a p f) w -> p a f w", a=2, f=4),
                                )
                                nc.scalar.dma_start(
                                    out=out_d[s][b, cq * 1024:(cq + 1) * 1024, :]
                                    .rearrange("(a p f) w -> p a f w", a=2, f=4),
                                    in_=iot,
                                )
                else:
                    Vi0 = pool_group(0)
                    load_consts2(nc.sync)
                    mamba_group(0, Vi0)
                    load_w(0, eng=nc.sync)
                    for g in range(NG):
                        ffn_pair(g, 0)
                        if g + 1 < NG:
                            Vi_g = pool_group(g + 1)
                            mamba_group(g + 1, Vi_g)
                        if g == 0:
                            load_w(1)
                    for g in range(NG):
                        ffn_pair(g, 1)

    nc.compile()
    return nc


# channel permutation: K-tile kc, partition p holds channel (kc//4)*512 + 4*p + (kc%4)
# so each DMA descriptor covers 4 consecutive channels (4.6KB contiguous).
_PERM = np.array(
    [[(kc // 4) * 512 + 4 * p + (kc % 4) for p in range(128)] for kc in range(NCT)]
).reshape(-1)  # [2048] in (kc, p) order


def _host_prep(inputs):
    """Host-side weight layout prep. Returns dict of per-core-replicated arrays."""
    f32 = np.float32
    g = lambda k: np.asarray(inputs[k], dtype=f32)
    s_bn = f32(1.0 / np.sqrt(1.0 + EPS))

    def ctile(v):  # [C] -> [128, 16], channel-permuted
        return np.ascontiguousarray(v[_PERM].reshape(NCT, 128).T)

    def dtile(v):  # [DI] -> [128, 2]
        return np.ascontiguousarray(v.reshape(DIT, 128).T)

    A = -np.exp(g("A_log"))  # [256, 16]
    W_in = g("W_in")
    Wf = W_in * g("ln1_g")[None, :]           # fold ln1 gain into W_in columns
    nwg_v = -(W_in @ g("ln1_g"))              # [256]
    wb_v = W_in @ g("ln1_b")                  # [256]
    sm_parts = {
        "wx": g("Wx").T.reshape(DIT, 128, 48).transpose(1, 0, 2).reshape(128, -1),
        "cw": g("conv_w")[:, 0, :].reshape(DIT, 128, 3).transpose(1, 0, 2).reshape(128, -1),
        "ncb": dtile(-g("conv_b")),
        "bdt": dtile(g("bdt")),
        "dssm": dtile(g("D_ssm")),
        "A3": A.reshape(DIT, 128, DS).transpose(1, 0, 2).reshape(128, -1),
        "nwg": dtile(nwg_v),
        "wb": dtile(wb_v),
        "ln2g": ctile(g("ln2_g")), "ln2b": ctile(g("ln2_b")),
        "absx": (g("att_bn_g") * s_bn)[:, None],
        "abnb": g("att_bn_b")[:, None],
        "fvs": ctile(g("ffn_vis_bn_g") * s_bn),
        "fvb": ctile(g("ffn_vis_b") * (g("ffn_vis_bn_g") * s_bn) + g("ffn_vis_bn_b")),
        "fis": ctile(g("ffn_inf_bn_g") * s_bn),
        "fib": ctile(g("ffn_inf_b") * (g("ffn_inf_bn_g") * s_bn) + g("ffn_inf_bn_b")),
    }
    smalls = np.zeros((128, SM_COLS), f32)
    for name, _w in SMALLS:
        a, b = SM_OFF[name]
        smalls[:, a:b] = sm_parts[name]

    prep = {
        "smalls": smalls,
        "w_inT": np.ascontiguousarray(
            Wf.T[_PERM].reshape(NCT, 128, DI).transpose(1, 0, 2)
        ).astype(ml_dtypes.bfloat16),
        "wdtT": np.ascontiguousarray(g("Wdt").T),
        "w_outT": np.ascontiguousarray(
            g("W_out").T[:, _PERM].reshape(DIT, 128, C).transpose(1, 0, 2)
        ).astype(ml_dtypes.bfloat16),
        "aw1T": np.ascontiguousarray(
            g("att_w1").T[_PERM].reshape(NCT, 128, 128).transpose(1, 0, 2)
        ).astype(ml_dtypes.bfloat16),
        "aw2T": np.ascontiguousarray(g("att_w2").T[:, _PERM]).astype(
            ml_dtypes.bfloat16
        ),
        "wvT": np.ascontiguousarray(
            g("ffn_vis_w").T[_PERM][:, _PERM].reshape(NCT, 128, C).transpose(1, 0, 2)
        ).astype(ml_dtypes.bfloat16),
        "wiT": np.ascontiguousarray(
            g("ffn_inf_w").T[_PERM][:, _PERM].reshape(NCT, 128, C).transpose(1, 0, 2)
        ).astype(ml_dtypes.bfloat16),
    }
    return prep


def _get_runner():
    """Build the bass program once and wrap it in a reusable jitted callable."""
    if "runner" in _CACHE:
        return _CACHE["runner"]

    import jax
    import numpy as _np
    from jax.sharding import Mesh, PartitionSpec
    from jax.experimental.shard_map import shard_map
    import concourse.bacc as bacc
    import concourse.tile as tile
    from concourse import mybir, masks
    from concourse import bass2jax

    nc = _build(bacc, tile, mybir, masks)
    bass2jax.install_neuronx_cc_hook()

    pname = nc.partition_id_tensor.name if nc.partition_id_tensor else None
    in_names, out_names, out_avals, zero_shapes = [], [], [], []
    for alloc in nc.m.functions[0].allocations:
        if not isinstance(alloc, mybir.MemoryLocationSet):
            continue
        name = alloc.memorylocations[0].name
        if alloc.kind == "ExternalInput":
            if name != pname:
                in_names.append(name)
        elif alloc.kind == "ExternalOutput":
            out_names.append(name)
            shape = tuple(alloc.tensor_shape)
            dtype = mybir.dt.np(alloc.dtype)
            out_avals.append(jax.core.ShapedArray(shape, dtype))
            zero_shapes.append((shape, dtype))
    n_params = len(in_names)
    all_names = list(in_names) + list(out_names)
    if pname is not None:
        all_names.append(pname)

    def _body(*args):
        operands = list(args)
        if pname is not None:
            operands.append(bass2jax.partition_id_tensor())
        outs = bass2jax._bass_exec_p.bind(
            *operands,
            out_avals=tuple(out_avals),
            in_names=tuple(all_names),
            out_names=tuple(out_names),
            lowering_input_output_aliases=(),
            sim_require_finite=False,
            sim_require_nnan=False,
            nc=nc,
        )
        return tuple(outs)

    devices = jax.devices()[:N_CORES]
    mesh = Mesh(_np.asarray(devices), ("core",))
    specs = (PartitionSpec("core"),) * (n_params + len(out_names))
    fn = jax.jit(
        shard_map(
            _body,
            mesh=mesh,
            in_specs=specs,
            out_specs=(PartitionSpec("core"),) * len(out_names),
            check_rep=False,
        ),
        keep_unused=True,
    )
    runner = {
        "fn": fn,
        "in_names": in_names,
        "out_names": out_names,
        "zero_shapes": zero_shapes,
        "nc": nc,
    }
    _CACHE["runner"] = runner
    return runner


def kernel(**inputs):
    runner = _get_runner()
    prep = _host_prep(inputs)
    vis = np.asarray(inputs["vis_feat_map"], dtype=np.float32).reshape(B_FULL, C, HW)
    inf = np.asarray(inputs["inf_feat_map"], dtype=np.float32).reshape(B_FULL, C, HW)

    # global inputs: concat of per-core shards along axis 0
    per_in = {"vis": vis, "inf": inf}  # already [64, ...] = 8 cores x [8, ...]
    gin = []
    for name in runner["in_names"]:
        if name in per_in:
            gin.append(per_in[name])
        else:
            arr = prep[name]
            gin.append(np.broadcast_to(arr, (N_CORES,) + arr.shape).reshape(
                (N_CORES * arr.shape[0],) + arr.shape[1:]
            ))
    zeros = [
        np.zeros((N_CORES * s[0],) + tuple(s[1:]), dt)
        for (s, dt) in runner["zero_shapes"]
    ]
    outs = runner["fn"](*gin, *zeros)
    res = {
        name: np.asarray(outs[i]) for i, name in enumerate(runner["out_names"])
    }
    out_vis = res["out_vis"].reshape(B_FULL, C, H, W)
    out_inf = res["out_inf"].reshape(B_FULL, C, H, W)
    return (out_vis, out_inf)


# revision 32
# speedup vs baseline: 1.0272x; 1.0272x over previous
"""Trainium2 Bass kernel for nn_CS_MAMBA (pool -> mamba -> channel-attention -> FFN).

Data-parallel over batch: 64 batch items sharded 8-per-core across 8 NeuronCores;
all weights replicated. Per core the 8 batch items are processed in 4 groups of
GB=2: the pool+mamba front-end of group g+1 overlaps the FFN matmuls of group g.
Column convention throughout the front-end: col = j*L + l (batch-in-group outer,
sequence INNER) so the selective scan is a single tensor_tensor_scan.
FFN matmuls pair the two batches of a group into 576-col moving operands.
"""

import numpy as np
import ml_dtypes

# ---------------------------------------------------------------- constants
B_FULL = 64
N_CORES = 8
BL = B_FULL // N_CORES          # 8 batch items per core
GB = 4                          # batch-group size
NG = BL // GB                   # 4 groups
C = 2048
NCT = C // 128                  # 16 channel tiles
H, W = 24, 12
HW = H * W                      # 288
POOL_W = 48                     # elements summed per patch (4 rows x 12 cols)
L = 12                          # interleaved sequence length
COLSG = L * GB                  # 24 group-local columns, col = j*L + l
DI = 256                        # d_inner
DIT = DI // 128                 # 2 d_inner tiles
DS = 16                         # d_state
DTR = 16                        # dt_rank
EPS = 1e-5

# packed per-partition small constants: name -> number of [128, n] columns
SMALLS = [
    ("wx", DIT * 48),       # Wx.T as [128, 2, 48]
    ("cw", DIT * 3),        # conv w as [128, 2, 3]
    ("ncb", DIT),           # -conv_b
    ("bdt", DIT),
    ("dssm", DIT),
    ("A3", DIT * DS),       # -exp(A_log) as [128, 2, 16]
    ("nwg", DIT),           # -(W_in @ ln1_g) as [128, 2]
    ("wb", DIT),            # W_in @ ln1_b as [128, 2]
    ("ln2g", NCT), ("ln2b", NCT),
    ("absx", 1), ("abnb", 1),
    ("fvs", NCT), ("fvb", NCT), ("fis", NCT), ("fib", NCT),
]
SM_OFF = {}
_off = 0
for _n, _w in SMALLS:
    SM_OFF[_n] = (_off, _off + _w)
    _off += _w
SM_COLS = _off

_CACHE = {}


def _build(nc_mod, tile_mod, mybir, masks, repeat=1, parts="all"):
    """Emit the bass program. Returns the compiled Bass object."""
    F32 = mybir.dt.float32
    BF16 = mybir.dt.bfloat16
    AF = mybir.ActivationFunctionType
    ALU = mybir.AluOpType
    AX = mybir.AxisListType

    nc = nc_mod.Bacc("TRN2", target_bir_lowering=False, debug=False)

    # ---------------- dram tensors (names = in_map keys)
    d_vis = nc.dram_tensor("vis", [BL, C, HW], F32, kind="ExternalInput")
    d_inf = nc.dram_tensor("inf", [BL, C, HW], F32, kind="ExternalInput")
    d_sm = nc.dram_tensor("smalls", [128, SM_COLS], F32, kind="ExternalInput")
    d_winT = nc.dram_tensor("w_inT", [128, NCT, DI], BF16, kind="ExternalInput")
    d_wdtT = nc.dram_tensor("wdtT", [DTR, DI], F32, kind="ExternalInput")
    d_woutT = nc.dram_tensor("w_outT", [128, DIT, C], BF16, kind="ExternalInput")
    d_aw1T = nc.dram_tensor("aw1T", [128, NCT, 128], BF16, kind="ExternalInput")
    d_aw2T = nc.dram_tensor("aw2T", [128, C], BF16, kind="ExternalInput")
    d_wvT = nc.dram_tensor("wvT", [128, NCT, C], BF16, kind="ExternalInput")
    d_wiT = nc.dram_tensor("wiT", [128, NCT, C], BF16, kind="ExternalInput")

    d_out_vis = nc.dram_tensor("out_vis", [BL, C, HW], F32, kind="ExternalOutput")
    d_out_inf = nc.dram_tensor("out_inf", [BL, C, HW], F32, kind="ExternalOutput")

    fm_d = [d_vis, d_inf]
    out_d = [d_out_vis, d_out_inf]

    with tile_mod.TileContext(nc) as tc:
        with (
            tc.tile_pool(name="consts", bufs=1) as consts,
            tc.tile_pool(name="wpool", bufs=1) as wpool,
            tc.tile_pool(name="stream", bufs=3) as stream,
            tc.tile_pool(name="axp", bufs=4) as axpool,
            tc.tile_pool(name="outp", bufs=2) as outp,
            tc.tile_pool(name="vip", bufs=1) as vip,
            tc.tile_pool(name="mam", bufs=1) as mam,
            tc.tile_pool(name="psA", bufs=4, space="PSUM") as psA,
            tc.tile_pool(name="psB", bufs=1, space="PSUM") as psB,
            tc.tile_pool(name="psC", bufs=2, space="PSUM") as psC,
        ):
            # ---------------- constants / weights to SBUF
            ident = consts.tile([128, 128], F32)
            masks.make_identity(nc, ident)
            ones_col = consts.tile([128, 1], F32)
            nc.vector.memset(ones_col, 1.0)
            ones_col_bf = consts.tile([128, 1], BF16)
            nc.vector.memset(ones_col_bf, 1.0)
            ones_row = consts.tile([1, 128], F32)
            nc.vector.memset(ones_row, 1.0)
            ones_row_bf = consts.tile([1, 128], BF16)
            nc.vector.memset(ones_row_bf, 1.0)
            eps1 = consts.tile([1, 1], F32)
            nc.vector.memset(eps1, EPS * POOL_W * POOL_W)   # LN1 stats on 48x sums
            eps2 = consts.tile([1, 1], F32)
            nc.vector.memset(eps2, EPS)

            sm = consts.tile([128, SM_COLS], F32)
            nc.gpsimd.dma_start(out=sm, in_=d_sm[:, :])

            def smv(name, i3=None):
                a, b = SM_OFF[name]
                v = sm[:, a:b]
                if i3 is not None:
                    v = v.rearrange("p (i k) -> p i k", i=i3)
                return v

            wxT = smv("wx", DIT)
            cw3 = smv("cw", DIT)
            ncb = smv("ncb")
            bdt2 = smv("bdt")
            dssm2 = smv("dssm")
            A3 = smv("A3", DIT)
            nwg = smv("nwg")
            wb2 = smv("wb")
            ln2g, ln2b = smv("ln2g"), smv("ln2b")
            absx, abnb = smv("absx"), smv("abnb")
            fvs, fvb = smv("fvs"), smv("fvb")
            fis, fib = smv("fis"), smv("fib")

            winT = consts.tile([128, NCT, DI], BF16)
            nc.gpsimd.dma_start(out=winT, in_=d_winT[:, :, :])
            wdtT = consts.tile([DTR, DI], F32)
            nc.gpsimd.dma_start(out=wdtT, in_=d_wdtT[:, :])
            woutT = consts.tile([128, DIT, C], BF16)
            aw1T = consts.tile([128, NCT, 128], BF16)
            aw2T = consts.tile([128, C], BF16)

            def load_consts2(eng):
                eng.dma_start(out=woutT, in_=d_woutT[:, :, :])
                eng.dma_start(out=aw1T, in_=d_aw1T[:, :, :])
                eng.dma_start(out=aw2T, in_=d_aw2T[:, :])

            import contextlib
            rep_ctx = tc.For_i(0, repeat, 1) if repeat > 1 else contextlib.nullcontext()
            with rep_ctx:
                # attention per group: [128, NCT, 2(s), GB(j)]
                att_g = [
                    consts.tile([128, NCT, 2, GB], F32, name=f"att{g}")
                    for g in range(NG)
                ]

                # ============================================================
                # Front-end. col = j*L + l, l = 2*pp + s.
                # ============================================================
                def pool_group(g):
                    Vi = vip.tile([128, NCT, COLSG], F32, tag="vi", bufs=NG,
                                  name=f"Vi{g}")
                    for j in range(GB):
                        b = g * GB + j
                        for s in range(2):
                            for cq in range(2):
                                ft = stream.tile(
                                    [128, 8, HW], F32, tag="fm",
                                    name=f"pfm{g}_{s}_{j}_{cq}",
                                )
                                nc.sync.dma_start(
                                    out=ft,
                                    in_=fm_d[s][b, cq * 1024:(cq + 1) * 1024, :]
                                    .rearrange("(a p f) w -> p a f w", a=2, f=4),
                                )
                                # windowed sums -> Vi[:, kc, j*L + 2*pp + s]
                                nc.vector.reduce_sum(
                                    out=Vi[:, cq * 8:(cq + 1) * 8, :].rearrange(
                                        "p c (j pp two) -> p c j pp two",
                                        j=GB, two=2,
                                    )[:, :, j, :, s],
                                    in_=ft.rearrange(
                                        "p c (pp w) -> p c pp w", w=POOL_W
                                    ),
                                    axis=AX.X,
                                )
                    return Vi

                def mamba_group(g, Vi):
                    # ---- LN1 stats on Vi (48x scale; eps1 compensates)
                    sq = mam.tile([128, NCT, COLSG], BF16, tag="lnsq", name="sq")
                    nc.scalar.activation(out=sq[:, :, :], in_=Vi[:, :, :],
                                         func=AF.Square)
                    vib = mam.tile([128, NCT, COLSG], BF16, tag="vib", name="vib")
                    nc.scalar.activation(out=vib[:, :, :], in_=Vi[:, :, :],
                                         func=AF.Copy)
                    s1p = psC.tile([128, COLSG], F32, tag="psS", name=f"s1p{g}")
                    s2p = psC.tile([128, COLSG], F32, tag="psS", name=f"s2p{g}")
                    for ci in range(NCT):
                        nc.tensor.matmul(
                            s1p[0:1, :], ones_col, Vi[:, ci, :],
                            start=(ci == 0), stop=(ci == NCT - 1),
                        )
                        nc.tensor.matmul(
                            s2p[0:1, :], ones_col_bf, sq[:, ci, :],
                            start=(ci == 0), stop=(ci == NCT - 1),
                        )

                    def ln_stats(s1, s2, epsv, tag):
                        m_sb = mam.tile([1, COLSG], F32, tag=f"m{tag}", name="m")
                        nc.vector.tensor_scalar_mul(m_sb, s1[0:1, :], 1.0 / C)
                        v_sb = mam.tile([1, COLSG], F32, tag=f"v{tag}", name="v")
                        nc.vector.tensor_scalar_mul(v_sb, s2[0:1, :], 1.0 / C)
                        msq = mam.tile([1, COLSG], F32, tag=f"msq{tag}", name="msq")
                        nc.vector.tensor_mul(msq, m_sb, m_sb)
                        nc.vector.tensor_sub(v_sb, v_sb, msq)
                        r_sb = mam.tile([1, COLSG], F32, tag=f"r{tag}", name="r")
                        nc.scalar.activation(out=r_sb, in_=v_sb, func=AF.Ln,
                                             bias=epsv)
                        nc.scalar.activation(out=r_sb, in_=r_sb, func=AF.Exp,
                                             scale=-0.5)
                        mr_sb = mam.tile([1, COLSG], F32, tag=f"mr{tag}", name="mr")
                        nc.vector.tensor_mul(mr_sb, m_sb, r_sb)
                        rbm = psC.tile([128, 2, COLSG], F32, tag="psS",
                                       name=f"rbm{tag}{g}")
                        nc.tensor.matmul(rbm[:, 0, :], ones_row, r_sb,
                                         start=True, stop=True)
                        nc.tensor.matmul(rbm[:, 1, :], ones_row, mr_sb,
                                         start=True, stop=True)
                        return rbm[:, 0, :], rbm[:, 1, :]

                    rb1, mrb1 = ln_stats(s1p, s2p, eps1, "a")

                    # ---- P = W_in' @ Vi ; x1 = rb*P + mrb*nwg + wb
                    Pp = psC.tile([128, DIT, COLSG], F32, tag="psS", name=f"Pp{g}")
                    for i in range(DIT):
                        for ci in range(NCT):
                            nc.tensor.matmul(
                                Pp[:, i, :], winT[:, ci, i * 128:(i + 1) * 128],
                                vib[:, ci, :],
                                start=(ci == 0), stop=(ci == NCT - 1),
                            )
                    x1 = mam.tile([128, DIT, COLSG], F32, tag="x1", name=f"x1{g}")
                    rb1_bc = rb1.unsqueeze(1).broadcast_to([128, DIT, COLSG])
                    nc.vector.tensor_copy(out=x1[:, :, :], in_=Pp[:, :, :])
                    nc.vector.tensor_tensor(out=x1[:, :, :], in0=x1[:, :, :],
                                            in1=rb1_bc, op=ALU.mult)
                    for i in range(DIT):
                        nc.vector.scalar_tensor_tensor(
                            out=x1[:, i, :], in0=mrb1[:, :], scalar=nwg[:, i:i + 1],
                            in1=x1[:, i, :], op0=ALU.mult, op1=ALU.add,
                        )
                        nc.vector.tensor_scalar_add(
                            out=x1[:, i, :], in0=x1[:, i, :],
                            scalar1=wb2[:, i:i + 1],
                        )

                    # ---- depthwise conv (along l, guarded per j) + silu
                    cv = mam.tile([128, DIT, COLSG], F32, tag="cv", name=f"cv{g}")
                    e_t = mam.tile([128, DIT, COLSG], F32, tag="e_t", name=f"e_t{g}")
                    xact = mam.tile([128, DIT, COLSG], F32, tag="xact",
                                    name=f"xact{g}")
                    for i in range(DIT):
                        nc.vector.tensor_scalar_mul(
                            out=cv[:, i, :], in0=x1[:, i, :], scalar1=cw3[:, i, 1:2]
                        )
                        cvj = cv[:, i, :].rearrange("p (j l) -> p j l", j=GB)
                        x1j = x1[:, i, :].rearrange("p (j l) -> p j l", j=GB)
                        nc.vector.scalar_tensor_tensor(
                            out=cvj[:, :, 1:L], in0=x1j[:, :, 0:L - 1],
                            scalar=cw3[:, i, 0:1], in1=cvj[:, :, 1:L],
                            op0=ALU.mult, op1=ALU.add,
                        )
                        nc.vector.scalar_tensor_tensor(
                            out=cvj[:, :, 0:L - 1], in0=x1j[:, :, 1:L],
                            scalar=cw3[:, i, 2:3], in1=cvj[:, :, 0:L - 1],
                            op0=ALU.mult, op1=ALU.add,
                        )
                        # silu(cv + cb) = (cv+cb)/(1+exp(-(cv+cb))); ncb = -conv_b
                        nc.scalar.activation(
                            out=e_t[:, i, :], in_=cv[:, i, :], func=AF.Exp,
                            scale=-1.0, bias=ncb[:, i:i + 1],
                        )
                    nc.vector.tensor_scalar_add(out=e_t[:, :, :], in0=e_t[:, :, :],
                                                scalar1=1.0)
                    nc.vector.reciprocal(out=e_t[:, :, :], in_=e_t[:, :, :])
                    for i in range(DIT):
                        nc.vector.scalar_tensor_tensor(
                            out=xact[:, i, :], in0=cv[:, i, :],
                            scalar=ncb[:, i:i + 1],
                            in1=e_t[:, i, :], op0=ALU.subtract, op1=ALU.mult,
                        )

                    # ---- dbc = x @ Wx.T -> [col, 48]; transpose -> [48, col]
                    dbcp = psC.tile([128, 48], F32, tag="psS", name=f"dbcp{g}")
                    for i in range(DIT):
                        nc.tensor.matmul(
                            dbcp[0:COLSG, :], xact[:, i, :], wxT[:, i, :],
                            start=(i == 0), stop=(i == DIT - 1),
                        )
                    dbc_sb = mam.tile([COLSG, 48], F32, tag="dbc", name=f"dbc{g}")
                    nc.vector.tensor_copy(out=dbc_sb, in_=dbcp[0:COLSG, :])
                    dtp = psC.tile([128, COLSG], F32, tag="psS", name=f"dtp{g}")
                    nc.tensor.transpose(
                        dtp[0:48, :], dbc_sb[:, :], ident[0:COLSG, 0:COLSG]
                    )
                    dbcT = mam.tile([48, COLSG], F32, tag="dbcT", name=f"dbcT{g}")
                    nc.vector.tensor_copy(out=dbcT, in_=dtp[0:48, :])

                    # ---- delta = softplus(dT @ Wdt.T + bdt) -> [256, col]
                    delta = mam.tile([128, DIT, COLSG], F32, tag="delta",
                                     name=f"delta{g}")
                    for i in range(DIT):
                        dp = psC.tile([128, COLSG], F32, tag="psS", name=f"dp{g}_{i}")
                        nc.tensor.matmul(
                            dp, wdtT[:, i * 128:(i + 1) * 128], dbcT[0:DTR, :],
                            start=True, stop=True,
                        )
                        nc.scalar.activation(
                            out=delta[:, i, :], in_=dp, func=AF.Exp,
                            bias=bdt2[:, i:i + 1],
                        )
                    nc.scalar.activation(out=delta[:, :, :], in_=delta[:, :, :],
                                         func=AF.Ln, bias=1.0)

                    # ---- Bp/Cp row-broadcast: flat [1, (n j l)] -> [128, (n j l)]
                    bcT_bf = mam.tile([48, COLSG], BF16, tag="bcTb",
                                      name=f"bcTb{g}")
                    nc.vector.tensor_copy(out=bcT_bf, in_=dtp[0:48, :])
                    bc_flat = mam.tile([1, 2 * DS * COLSG], BF16, tag="flat",
                                       name=f"bcf{g}")
                    nc.gpsimd.dma_start(
                        out=bc_flat.rearrange("o (n c) -> o n c", n=2 * DS),
                        in_=bcT_bf[DTR:DTR + 2 * DS, :],
                    )
                    NBC = DS * COLSG
                    bpb = psB.tile([128, NBC], F32, tag="big", bufs=1, name=f"bpb{g}")
                    cpb_ps = psB.tile([128, NBC], F32, tag="big", bufs=1,
                                      name=f"cpp{g}")
                    for k in range((NBC + 511) // 512):
                        sl = slice(512 * k, min(NBC, 512 * (k + 1)))
                        nc.tensor.matmul(bpb[:, sl], ones_row_bf, bc_flat[:, sl],
                                         start=True, stop=True)
                        nc.tensor.matmul(
                            cpb_ps[:, sl], ones_row_bf,
                            bc_flat[:, NBC + 512 * k:NBC + sl.stop],
                            start=True, stop=True)
                    cpb = mam.tile([128, DS * COLSG], BF16, tag="cpbs", name=f"cpb{g}")
                    nc.vector.tensor_copy(out=cpb, in_=cpb_ps)

                    # ---- dA = exp(delta x A), memory order [128, (i n j l)]
                    dA = mam.tile([128, DIT, DS * COLSG], BF16, tag="dA",
                                  name=f"dA{g}")
                    for i in range(DIT):
                        nc.vector.tensor_tensor(
                            out=dA[:, i, :].rearrange("p (n c) -> p n c", n=DS),
                            in0=delta[:, i, :].unsqueeze(1)
                            .broadcast_to([128, DS, COLSG]),
                            in1=A3[:, i, :].unsqueeze(2)
                            .broadcast_to([128, DS, COLSG]),
                            op=ALU.mult,
                        )
                    nc.scalar.activation(out=dA[:, :, :], in_=dA[:, :, :],
                                         func=AF.Exp)
                    # zero dA at l==0 of every (i,n,j) block -> scan resets there
                    nc.vector.tensor_scalar_mul(
                        out=dA.rearrange("p i (b l) -> p (i b) l", l=L)[:, :, 0:1],
                        in0=dA.rearrange("p i (b l) -> p (i b) l", l=L)[:, :, 0:1],
                        scalar1=0.0,
                    )

                    # ---- dBu = (delta*x) x Bp
                    du = mam.tile([128, DIT, COLSG], F32, tag="du", name=f"du{g}")
                    nc.vector.tensor_mul(du[:, :, :], delta[:, :, :], xact[:, :, :])
                    dBu = mam.tile([128, DIT, DS * COLSG], BF16, tag="dBu",
                                   name=f"dBu{g}")
                    for i in range(DIT):
                        nc.vector.tensor_tensor(
                            out=dBu[:, i, :].rearrange("p (n c) -> p n c", n=DS),
                            in0=du[:, i, :].unsqueeze(1)
                            .broadcast_to([128, DS, COLSG]),
                            in1=bpb.rearrange("p (n c) -> p n c", n=DS),
                            op=ALU.mult,
                        )

                    # ---- selective scan: one pass over (i n j l), l innermost
                    Hs = mam.tile([128, DIT * DS * COLSG], BF16, tag="Hs",
                                  name=f"Hs{g}")
                    nc.vector.tensor_tensor_scan(
                        out=Hs[:, :],
                        data0=dA.rearrange("p i c -> p (i c)"),
                        data1=dBu.rearrange("p i c -> p (i c)"),
                        initial=0.0, op0=ALU.mult, op1=ALU.add,
                    )

                    # ---- y = sum_n H * Cp  (+ x * D_ssm)
                    tt = mam.tile([128, DIT, DS * COLSG], BF16, tag="dA",
                                  name=f"yt{g}")
                    nc.vector.tensor_tensor(
                        out=tt[:, :, :],
                        in0=Hs.rearrange("p (i c) -> p i c", i=DIT),
                        in1=cpb.unsqueeze(1).broadcast_to([128, DIT, DS * COLSG]),
                        op=ALU.mult,
                    )
                    y2 = mam.tile([128, DIT, COLSG], F32, tag="y2", name=f"y2{g}")
                    nc.vector.reduce_sum(
                        out=y2[:, :, :],
                        in_=tt.rearrange("p i (n c) -> p i c n", n=DS),
                        axis=AX.X,
                    )
                    for i in range(DIT):
                        nc.vector.scalar_tensor_tensor(
                            out=y2[:, i, :], in0=xact[:, i, :],
                            scalar=dssm2[:, i:i + 1],
                            in1=y2[:, i, :], op0=ALU.mult, op1=ALU.add,
                        )
                    y2b = mam.tile([128, DIT, COLSG], BF16, tag="y2b", name=f"y2b{g}")
                    nc.scalar.activation(out=y2b[:, :, :], in_=y2[:, :, :],
                                         func=AF.Copy)

                    # ---- vi2 = y2 @ W_out.T + Vi/48
                    vi2 = vip.tile([128, NCT, COLSG], F32, tag="v24", bufs=2,
                                   name=f"vi2{g}")
                    vi2p = psB.tile([128, NCT, 64], F32, tag="big", bufs=1,
                                    name=f"vi2p{g}")
                    for mc in range(NCT):
                        for i in range(DIT):
                            nc.tensor.matmul(
                                vi2p[:, mc, 0:COLSG],
                                woutT[:, i, mc * 128:(mc + 1) * 128], y2b[:, i, :],
                                start=(i == 0), stop=(i == DIT - 1),
                            )
                    nc.vector.scalar_tensor_tensor(
                        out=vi2[:, :, :], in0=Vi[:, :, :], scalar=1.0 / POOL_W,
                        in1=vi2p[:, :, 0:COLSG], op0=ALU.mult, op1=ALU.add,
                    )

                    # ---- LN2 (generic g/b)
                    sq2 = mam.tile([128, NCT, COLSG], BF16, tag="lnsq", name="sq2")
                    nc.scalar.activation(out=sq2[:, :, :], in_=vi2[:, :, :],
                                         func=AF.Square)
                    t1p = psC.tile([128, COLSG], F32, tag="psS", name=f"t1p{g}")
                    t2p = psC.tile([128, COLSG], F32, tag="psS", name=f"t2p{g}")
                    for ci in range(NCT):
                        nc.tensor.matmul(
                            t1p[0:1, :], ones_col, vi2[:, ci, :],
                            start=(ci == 0), stop=(ci == NCT - 1),
                        )
                        nc.tensor.matmul(
                            t2p[0:1, :], ones_col_bf, sq2[:, ci, :],
                            start=(ci == 0), stop=(ci == NCT - 1),
                        )
                    rb2, mrb2 = ln_stats(t1p, t2p, eps2, "b")
                    nrm = vip.tile([128, NCT, COLSG], F32, tag="v24", bufs=2,
                                   name=f"nrm{g}")
                    rb2_bc = rb2.unsqueeze(1).broadcast_to([128, NCT, COLSG])
                    mrb2_bc = mrb2.unsqueeze(1).broadcast_to([128, NCT, COLSG])
                    g_bc = ln2g.unsqueeze(2).broadcast_to([128, NCT, COLSG])
                    b_bc = ln2b.unsqueeze(2).broadcast_to([128, NCT, COLSG])
                    nc.vector.tensor_tensor(out=nrm[:, :, :], in0=vi2[:, :, :],
                                            in1=rb2_bc, op=ALU.mult)
                    nc.vector.tensor_tensor(out=nrm[:, :, :], in0=nrm[:, :, :],
                                            in1=mrb2_bc, op=ALU.subtract)
                    nc.vector.tensor_tensor(out=nrm[:, :, :], in0=nrm[:, :, :],
                                            in1=g_bc, op=ALU.mult)
                    nc.vector.tensor_tensor(out=nrm[:, :, :], in0=nrm[:, :, :],
                                            in1=b_bc, op=ALU.add)

                    # ---- channel attention, both streams + mean/max merged.
                    # mvx cols per c: (src(2), s(2), j(2)); col = j*L + 2*pp + s
                    mvx = mam.tile([128, NCT, 4 * GB], F32, tag="mvx",
                                   name=f"mvx{g}")
                    mvx5 = mvx.rearrange("p c (x s j) -> p c x s j", x=2, s=2)
                    nv = nrm.rearrange("p c (j pp two) -> p c two j pp", j=GB,
                                       two=2)
                    for s in range(2):
                        nc.vector.reduce_sum(out=mvx5[:, :, 0, s, :],
                                             in_=nv[:, :, s, :, :], axis=AX.X)
                        nc.vector.reduce_max(out=mvx5[:, :, 1, s, :],
                                             in_=nv[:, :, s, :, :], axis=AX.X)
                    # mean path: sum/6
                    nc.vector.tensor_scalar_mul(
                        out=mvx.rearrange("p c (x sj) -> p c x sj", x=2)[:, :, 0, :],
                        in0=mvx.rearrange("p c (x sj) -> p c x sj", x=2)[:, :, 0, :],
                        scalar1=1.0 / (L // 2),
                    )
                    mvxb = mam.tile([128, NCT, 4 * GB], BF16, tag="mvxb",
                                    name=f"mvxb{g}")
                    nc.scalar.activation(out=mvxb[:, :, :],
                                         in_=mvx[:, :, :], func=AF.Copy)
                    hp = psC.tile([128, COLSG], F32, tag="psS", name=f"hp{g}")
                    for ci in range(NCT):
                        nc.tensor.matmul(
                            hp[:, 0:4 * GB], aw1T[:, ci, :],
                            mvxb[:, ci, :],
                            start=(ci == 0), stop=(ci == NCT - 1),
                        )
                    h1 = mam.tile([128, 2, 2 * GB], BF16, tag="h1", name=f"h1{g}")
                    nc.scalar.activation(
                        out=h1[:, :, :], in_=hp[:, 0:4 * GB], func=AF.Relu,
                        scale=absx, bias=abnb,
                    )
                    h1s = mam.tile([128, 2 * GB], BF16, tag="h1s", name=f"h1s{g}")
                    nc.vector.tensor_tensor(out=h1s, in0=h1[:, 0, :],
                                            in1=h1[:, 1, :], op=ALU.add)
                    att = att_g[g]
                    apb = psB.tile([128, NCT, 2 * GB], F32, tag="big", bufs=1,
                                   name=f"apb{g}")
                    for mc in range(NCT):
                        nc.tensor.matmul(
                            apb[:, mc, :], aw2T[:, mc * 128:(mc + 1) * 128], h1s,
                            start=True, stop=True,
                        )
                    nc.scalar.activation(
                        out=att[:, :, :, :], in_=apb[:, :, :], func=AF.Exp,
                        scale=-1.0,
                    )
                    nc.vector.tensor_scalar_add(out=att[:, :, :, :],
                                                in0=att[:, :, :, :], scalar1=1.0)
                    nc.vector.reciprocal(out=att[:, :, :, :], in_=att[:, :, :, :])

                # ============================================================
                # FFN: per (group, stream) pair of batches, 576-col matmuls.
                # ============================================================
                scl_s = [fvs, fis]
                scl_b = [fvb, fib]
                w_dram = [d_wvT, d_wiT]
                wts = {}

                def load_w(s, eng=None):
                    eng = eng or nc.gpsimd
                    wt = [
                        wpool.tile([128, C], BF16, tag="w", bufs=17,
                                   name=f"w{s}_{kc}")
                        for kc in range(NCT)
                    ]
                    for kc in range(NCT):
                        eng.dma_start(out=wt[kc], in_=w_dram[s][:, kc, :])
                    wts[s] = wt

                def ffn_pair(g, s, skip_io=False):
                    wt = wts[s]
                    att = att_g[g]
                    for j in range(GB):
                        b = g * GB + j
                        axt = [
                            axpool.tile([128, 8, HW], BF16, tag="ax", bufs=4,
                                        name=f"ax{s}_{g}_{j}_{cq}")
                            for cq in range(2)
                        ]
                        if skip_io:
                            for cq in range(2):
                                nc.gpsimd.memset(axt[cq], 0.01)
                        else:
                            for cq in range(2):
                                ft = stream.tile(
                                    [128, 8, HW], F32, tag="fm",
                                    name=f"ffm{s}_{g}_{j}_{cq}",
                                )
                                nc.scalar.dma_start(
                                    out=ft,
                                    in_=fm_d[s][b, cq * 1024:(cq + 1) * 1024, :]
                                    .rearrange("(a p f) w -> p a f w", a=2, f=4),
                                )
                                if cq == 0:
                                    nc.vector.tensor_tensor(
                                        out=axt[cq][:, :, :],
                                        in0=ft[:, :, :],
                                        in1=att[:, 0:8, s, j]
                                        .unsqueeze(2).broadcast_to([128, 8, HW]),
                                        op=ALU.mult,
                                    )
                                else:
                                    for kl in range(8):
                                        nc.scalar.activation(
                                            out=axt[cq][:, kl, :],
                                            in_=ft[:, kl, :], func=AF.Copy,
                                            scale=att[:, 8 + kl, s, j:j + 1],
                                        )
                        for mq in range(NCT // 4):
                            ot = outp.tile(
                                [128, 4, HW], F32, tag="ot",
                                name=f"ot{s}_{g}_{j}_{mq}"
                            )
                            for mi in range(4):
                                mc = mq * 4 + mi
                                pp = psA.tile([128, HW], F32, tag="pp", bufs=4,
                                              name=f"pp{s}_{g}_{j}_{mc}")
                                for kc in range(NCT):
                                    nc.tensor.matmul(
                                        pp, wt[kc][:, mc * 128:(mc + 1) * 128],
                                        axt[kc // 8][:, kc % 8, :],
                                        start=(kc == 0), stop=(kc == NCT - 1),
                                    )
                                nc.scalar.activation(
                                    out=ot[:, mi, :], in_=pp, func=AF.Relu,
                                    scale=scl_s[s][:, mc:mc + 1],
                                    bias=scl_b[s][:, mc:mc + 1],
                                )
                            if not skip_io:
                                nc.scalar.dma_start(
                                    out=out_d[s][b, mq * 512:(mq + 1) * 512, :]
                                    .rearrange("(p four) w -> p four w", four=4),
                                    in_=ot,
                                )

                # ---- emission: software-pipelined fronts and FFN stages
                if parts != "all":
                    load_consts2(nc.gpsimd)
                if parts == "front":
                    for g in range(NG):
                        Vi_g = pool_group(g)
                        mamba_group(g, Vi_g)
                elif parts == "pool":
                    for g in range(NG):
                        pool_group(g)
                elif parts == "mamba":
                    for g in range(NG):
                        Vi_g = vip.tile([128, NCT, COLSG], F32, tag="vi", bufs=NG,
                                        name=f"Vi{g}")
                        nc.vector.memset(Vi_g, 0.5)
                        mamba_group(g, Vi_g)
                elif parts in ("ffn", "mm"):
                    for gg in range(NG):
                        nc.vector.memset(att_g[gg], 1.0)
                    load_w(0)
                    for g in range(NG):
                        ffn_pair(g, 0, skip_io=(parts == "mm"))
                    load_w(1)
                    for g in range(NG):
                        ffn_pair(g, 1, skip_io=(parts == "mm"))
                elif parts == "io":
                    iot = consts.tile([128, 8, HW], F32)
                    nc.vector.memset(iot, 0.25)
                    for s in range(2):
                        for b in range(BL):
                            for cq in range(2):
                                ft = stream.tile([128, 8, HW], F32, tag="fm",
                                                 name=f"ioi{s}_{b}_{cq}")
                                nc.sync.dma_start(
                                    out=ft,
                                    in_=fm_d[s][b, cq * 1024:(cq + 1) * 1024, :]
                                    .rearrange("(a p f) w -> p a f w", a=2, f=4),
                                )
                                nc.scalar.dma_start(
                                    out=out_d[s][b, cq * 1024:(cq + 1) * 1024, :]
                                    .rearrange("(a p f) w -> p a f w", a=2, f=4),
                                    in_=iot,
                                )
                else:
                    Vi0 = pool_group(0)
                    load_consts2(nc.sync)
                    mamba_group(0, Vi0)
                    load_w(0, eng=nc.sync)
                    for g in range(NG):
                        ffn_pair(g, 0)
                        if g + 1 < NG:
                            Vi_g = pool_group(g + 1)
                            mamba_group(g + 1, Vi_g)
                        if g == 0:
                            load_w(1)
                    for g in range(NG):
                        ffn_pair(g, 1)

    nc.compile()
    return nc


# channel permutation: K-tile kc, partition p holds channel (kc//4)*512 + 4*p + (kc%4)
# so each DMA descriptor covers 4 consecutive channels (4.6KB contiguous).
_PERM = np.array(
    [[(kc // 4) * 512 + 4 * p + (kc % 4) for p in range(128)] for kc in range(NCT)]
).reshape(-1)  # [2048] in (kc, p) order


def _host_prep(inputs):
    """Host-side weight layout prep. Returns dict of per-core-replicated arrays."""
    f32 = np.float32
    g = lambda k: np.asarray(inputs[k], dtype=f32)
    s_bn = f32(1.0 / np.sqrt(1.0 + EPS))

    def ctile(v):  # [C] -> [128, 16], channel-permuted
        return np.ascontiguousarray(v[_PERM].reshape(NCT, 128).T)

    def dtile(v):  # [DI] -> [128, 2]
        return np.ascontiguousarray(v.reshape(DIT, 128).T)

    A = -np.exp(g("A_log"))  # [256, 16]
    W_in = g("W_in")
    Wf = W_in * g("ln1_g")[None, :]           # fold ln1 gain into W_in columns
    nwg_v = -(W_in @ g("ln1_g"))              # [256]
    wb_v = W_in @ g("ln1_b")                  # [256]
    sm_parts = {
        "wx": g("Wx").T.reshape(DIT, 128, 48).transpose(1, 0, 2).reshape(128, -1),
        "cw": g("conv_w")[:, 0, :].reshape(DIT, 128, 3).transpose(1, 0, 2).reshape(128, -1),
        "ncb": dtile(-g("conv_b")),
        "bdt": dtile(g("bdt")),
        "dssm": dtile(g("D_ssm")),
        "A3": A.reshape(DIT, 128, DS).transpose(1, 0, 2).reshape(128, -1),
        "nwg": dtile(nwg_v),
        "wb": dtile(wb_v),
        "ln2g": ctile(g("ln2_g")), "ln2b": ctile(g("ln2_b")),
        "absx": (g("att_bn_g") * s_bn)[:, None],
        "abnb": g("att_bn_b")[:, None],
        "fvs": ctile(g("ffn_vis_bn_g") * s_bn),
        "fvb": ctile(g("ffn_vis_b") * (g("ffn_vis_bn_g") * s_bn) + g("ffn_vis_bn_b")),
        "fis": ctile(g("ffn_inf_bn_g") * s_bn),
        "fib": ctile(g("ffn_inf_b") * (g("ffn_inf_bn_g") * s_bn) + g("ffn_inf_bn_b")),
    }
    smalls = np.zeros((128, SM_COLS), f32)
    for name, _w in SMALLS:
        a, b = SM_OFF[name]
        smalls[:, a:b] = sm_parts[name]

    prep = {
        "smalls": smalls,
        "w_inT": np.ascontiguousarray(
            Wf.T[_PERM].reshape(NCT, 128, DI).transpose(1, 0, 2)
        ).astype(ml_dtypes.bfloat16),
        "wdtT": np.ascontiguousarray(g("Wdt").T),
        "w_outT": np.ascontiguousarray(
            g("W_out").T[:, _PERM].reshape(DIT, 128, C).transpose(1, 0, 2)
        ).astype(ml_dtypes.bfloat16),
        "aw1T": np.ascontiguousarray(
            g("att_w1").T[_PERM].reshape(NCT, 128, 128).transpose(1, 0, 2)
        ).astype(ml_dtypes.bfloat16),
        "aw2T": np.ascontiguousarray(g("att_w2").T[:, _PERM]).astype(
            ml_dtypes.bfloat16
        ),
        "wvT": np.ascontiguousarray(
            g("ffn_vis_w").T[_PERM][:, _PERM].reshape(NCT, 128, C).transpose(1, 0, 2)
        ).astype(ml_dtypes.bfloat16),
        "wiT": np.ascontiguousarray(
            g("ffn_inf_w").T[_PERM][:, _PERM].reshape(NCT, 128, C).transpose(1, 0, 2)
        ).astype(ml_dtypes.bfloat16),
    }
    return prep


def _get_runner():
    """Build the bass program once and wrap it in a reusable jitted callable."""
    if "runner" in _CACHE:
        return _CACHE["runner"]

    import jax
    import numpy as _np
    from jax.sharding import Mesh, PartitionSpec
    from jax.experimental.shard_map import shard_map
    import concourse.bacc as bacc
    import concourse.tile as tile
    from concourse import mybir, masks
    from concourse import bass2jax

    nc = _build(bacc, tile, mybir, masks)
    bass2jax.install_neuronx_cc_hook()

    pname = nc.partition_id_tensor.name if nc.partition_id_tensor else None
    in_names, out_names, out_avals, zero_shapes = [], [], [], []
    for alloc in nc.m.functions[0].allocations:
        if not isinstance(alloc, mybir.MemoryLocationSet):
            continue
        name = alloc.memorylocations[0].name
        if alloc.kind == "ExternalInput":
            if name != pname:
                in_names.append(name)
        elif alloc.kind == "ExternalOutput":
            out_names.append(name)
            shape = tuple(alloc.tensor_shape)
            dtype = mybir.dt.np(alloc.dtype)
            out_avals.append(jax.core.ShapedArray(shape, dtype))
            zero_shapes.append((shape, dtype))
    n_params = len(in_names)
    all_names = list(in_names) + list(out_names)
    if pname is not None:
        all_names.append(pname)

    def _body(*args):
        operands = list(args)
        if pname is not None:
            operands.append(bass2jax.partition_id_tensor())
        outs = bass2jax._bass_exec_p.bind(
            *operands,
            out_avals=tuple(out_avals),
            in_names=tuple(all_names),
            out_names=tuple(out_names),
            lowering_input_output_aliases=(),
            sim_require_finite=False,
            sim_require_nnan=False,
            nc=nc,
        )
        return tuple(outs)

    devices = jax.devices()[:N_CORES]
    mesh = Mesh(_np.asarray(devices), ("core",))
    specs = (PartitionSpec("core"),) * (n_params + len(out_names))
    fn = jax.jit(
        shard_map(
            _body,
            mesh=mesh,
            in_specs=specs,
            out_specs=(PartitionSpec("core"),) * len(out_names),
            check_rep=False,
        ),
        keep_unused=True,
    )
    runner = {
        "fn": fn,
        "in_names": in_names,
        "out_names": out_names,
        "zero_shapes": zero_shapes,
        "nc": nc,
    }
    _CACHE["runner"] = runner
    return runner


def kernel(**inputs):
    runner = _get_runner()
    prep = _host_prep(inputs)
    vis = np.asarray(inputs["vis_feat_map"], dtype=np.float32).reshape(B_FULL, C, HW)
    inf = np.asarray(inputs["inf_feat_map"], dtype=np.float32).reshape(B_FULL, C, HW)

    # global inputs: concat of per-core shards along axis 0
    per_in = {"vis": vis, "inf": inf}  # already [64, ...] = 8 cores x [8, ...]
    gin = []
    for name in runner["in_names"]:
        if name in per_in:
            gin.append(per_in[name])
        else:
            arr = prep[name]
            gin.append(np.broadcast_to(arr, (N_CORES,) + arr.shape).reshape(
                (N_CORES * arr.shape[0],) + arr.shape[1:]
            ))
    zeros = [
        np.zeros((N_CORES * s[0],) + tuple(s[1:]), dt)
        for (s, dt) in runner["zero_shapes"]
    ]
    outs = runner["fn"](*gin, *zeros)
    res = {
        name: np.asarray(outs[i]) for i, name in enumerate(runner["out_names"])
    }
    out_vis = res["out_vis"].reshape(B_FULL, C, H, W)
    out_inf = res["out_inf"].reshape(B_FULL, C, H, W)
    return (out_vis, out_inf)


# revision 33
# speedup vs baseline: 1.0353x; 1.0079x over previous
"""Trainium2 Bass kernel for nn_CS_MAMBA (pool -> mamba -> channel-attention -> FFN).

Data-parallel over batch: 64 batch items sharded 8-per-core across 8 NeuronCores;
all weights replicated. Per core the 8 batch items are processed in 4 groups of
GB=2: the pool+mamba front-end of group g+1 overlaps the FFN matmuls of group g.
Column convention throughout the front-end: col = j*L + l (batch-in-group outer,
sequence INNER) so the selective scan is a single tensor_tensor_scan.
FFN matmuls pair the two batches of a group into 576-col moving operands.
"""

import numpy as np
import ml_dtypes

# ---------------------------------------------------------------- constants
B_FULL = 64
N_CORES = 8
BL = B_FULL // N_CORES          # 8 batch items per core
GB = 4                          # batch-group size
NG = BL // GB                   # 4 groups
C = 2048
NCT = C // 128                  # 16 channel tiles
H, W = 24, 12
HW = H * W                      # 288
POOL_W = 48                     # elements summed per patch (4 rows x 12 cols)
L = 12                          # interleaved sequence length
COLSG = L * GB                  # 24 group-local columns, col = j*L + l
DI = 256                        # d_inner
DIT = DI // 128                 # 2 d_inner tiles
DS = 16                         # d_state
DTR = 16                        # dt_rank
EPS = 1e-5

# packed per-partition small constants: name -> number of [128, n] columns
SMALLS = [
    ("wx", DIT * 48),       # Wx.T as [128, 2, 48]
    ("cw", DIT * 3),        # conv w as [128, 2, 3]
    ("ncb", DIT),           # -conv_b
    ("bdt", DIT),
    ("dssm", DIT),
    ("A3", DIT * DS),       # -exp(A_log) as [128, 2, 16]
    ("nwg", DIT),           # -(W_in @ ln1_g) as [128, 2]
    ("wb", DIT),            # W_in @ ln1_b as [128, 2]
    ("ln2g", NCT), ("ln2b", NCT),
    ("absx", 1), ("abnb", 1),
    ("fvs", NCT), ("fvb", NCT), ("fis", NCT), ("fib", NCT),
]
SM_OFF = {}
_off = 0
for _n, _w in SMALLS:
    SM_OFF[_n] = (_off, _off + _w)
    _off += _w
SM_COLS = _off

_CACHE = {}


def _build(nc_mod, tile_mod, mybir, masks, repeat=1, parts="all"):
    """Emit the bass program. Returns the compiled Bass object."""
    F32 = mybir.dt.float32
    BF16 = mybir.dt.bfloat16
    AF = mybir.ActivationFunctionType
    ALU = mybir.AluOpType
    AX = mybir.AxisListType

    nc = nc_mod.Bacc("TRN2", target_bir_lowering=False, debug=False)

    # ---------------- dram tensors (names = in_map keys)
    d_vis = nc.dram_tensor("vis", [BL, C, HW], F32, kind="ExternalInput")
    d_inf = nc.dram_tensor("inf", [BL, C, HW], F32, kind="ExternalInput")
    d_sm = nc.dram_tensor("smalls", [128, SM_COLS], F32, kind="ExternalInput")
    d_winT = nc.dram_tensor("w_inT", [128, NCT, DI], BF16, kind="ExternalInput")
    d_wdtT = nc.dram_tensor("wdtT", [DTR, DI], F32, kind="ExternalInput")
    d_woutT = nc.dram_tensor("w_outT", [128, DIT, C], BF16, kind="ExternalInput")
    d_aw1T = nc.dram_tensor("aw1T", [128, NCT, 128], BF16, kind="ExternalInput")
    d_aw2T = nc.dram_tensor("aw2T", [128, C], BF16, kind="ExternalInput")
    d_wvT = nc.dram_tensor("wvT", [128, NCT, C], BF16, kind="ExternalInput")
    d_wiT = nc.dram_tensor("wiT", [128, NCT, C], BF16, kind="ExternalInput")

    d_out_vis = nc.dram_tensor("out_vis", [BL, C, HW], F32, kind="ExternalOutput")
    d_out_inf = nc.dram_tensor("out_inf", [BL, C, HW], F32, kind="ExternalOutput")

    fm_d = [d_vis, d_inf]
    out_d = [d_out_vis, d_out_inf]

    with tile_mod.TileContext(nc) as tc:
        with (
            tc.tile_pool(name="consts", bufs=1) as consts,
            tc.tile_pool(name="wpool", bufs=1) as wpool,
            tc.tile_pool(name="stream", bufs=3) as stream,
            tc.tile_pool(name="axp", bufs=4) as axpool,
            tc.tile_pool(name="outp", bufs=3) as outp,
            tc.tile_pool(name="vip", bufs=1) as vip,
            tc.tile_pool(name="mam", bufs=1) as mam,
            tc.tile_pool(name="psA", bufs=4, space="PSUM") as psA,
            tc.tile_pool(name="psB", bufs=1, space="PSUM") as psB,
            tc.tile_pool(name="psC", bufs=2, space="PSUM") as psC,
        ):
            # ---------------- constants / weights to SBUF
            ident = consts.tile([128, 128], F32)
            masks.make_identity(nc, ident)
            ones_col = consts.tile([128, 1], F32)
            nc.vector.memset(ones_col, 1.0)
            ones_col_bf = consts.tile([128, 1], BF16)
            nc.vector.memset(ones_col_bf, 1.0)
            ones_row = consts.tile([1, 128], F32)
            nc.vector.memset(ones_row, 1.0)
            ones_row_bf = consts.tile([1, 128], BF16)
            nc.vector.memset(ones_row_bf, 1.0)
            eps1 = consts.tile([1, 1], F32)
            nc.vector.memset(eps1, EPS * POOL_W * POOL_W)   # LN1 stats on 48x sums
            eps2 = consts.tile([1, 1], F32)
            nc.vector.memset(eps2, EPS)

            sm = consts.tile([128, SM_COLS], F32)
            nc.gpsimd.dma_start(out=sm, in_=d_sm[:, :])

            def smv(name, i3=None):
                a, b = SM_OFF[name]
                v = sm[:, a:b]
                if i3 is not None:
                    v = v.rearrange("p (i k) -> p i k", i=i3)
                return v

            wxT = smv("wx", DIT)
            cw3 = smv("cw", DIT)
            ncb = smv("ncb")
            bdt2 = smv("bdt")
            dssm2 = smv("dssm")
            A3 = smv("A3", DIT)
            nwg = smv("nwg")
            wb2 = smv("wb")
            ln2g, ln2b = smv("ln2g"), smv("ln2b")
            absx, abnb = smv("absx"), smv("abnb")
            fvs, fvb = smv("fvs"), smv("fvb")
            fis, fib = smv("fis"), smv("fib")

            winT = consts.tile([128, NCT, DI], BF16)
            nc.gpsimd.dma_start(out=winT, in_=d_winT[:, :, :])
            wdtT = consts.tile([DTR, DI], F32)
            nc.gpsimd.dma_start(out=wdtT, in_=d_wdtT[:, :])
            woutT = consts.tile([128, DIT, C], BF16)
            aw1T = consts.tile([128, NCT, 128], BF16)
            aw2T = consts.tile([128, C], BF16)

            def load_consts2(eng):
                eng.dma_start(out=woutT, in_=d_woutT[:, :, :])
                eng.dma_start(out=aw1T, in_=d_aw1T[:, :, :])
                eng.dma_start(out=aw2T, in_=d_aw2T[:, :])

            import contextlib
            rep_ctx = tc.For_i(0, repeat, 1) if repeat > 1 else contextlib.nullcontext()
            with rep_ctx:
                # attention per group: [128, NCT, 2(s), GB(j)]
                att_g = [
                    consts.tile([128, NCT, 2, GB], F32, name=f"att{g}")
                    for g in range(NG)
                ]

                # ============================================================
                # Front-end. col = j*L + l, l = 2*pp + s.
                # ============================================================
                def pool_group(g):
                    Vi = vip.tile([128, NCT, COLSG], F32, tag="vi", bufs=NG,
                                  name=f"Vi{g}")
                    for j in range(GB):
                        b = g * GB + j
                        for s in range(2):
                            for cq in range(2):
                                ft = stream.tile(
                                    [128, 8, HW], F32, tag="fm",
                                    name=f"pfm{g}_{s}_{j}_{cq}",
                                )
                                nc.sync.dma_start(
                                    out=ft,
                                    in_=fm_d[s][b, cq * 1024:(cq + 1) * 1024, :]
                                    .rearrange("(a p f) w -> p a f w", a=2, f=4),
                                )
                                # windowed sums -> Vi[:, kc, j*L + 2*pp + s]
                                nc.vector.reduce_sum(
                                    out=Vi[:, cq * 8:(cq + 1) * 8, :].rearrange(
                                        "p c (j pp two) -> p c j pp two",
                                        j=GB, two=2,
                                    )[:, :, j, :, s],
                                    in_=ft.rearrange(
                                        "p c (pp w) -> p c pp w", w=POOL_W
                                    ),
                                    axis=AX.X,
                                )
                    return Vi

                def mamba_group(g, Vi):
                    # ---- LN1 stats on Vi (48x scale; eps1 compensates)
                    sq = mam.tile([128, NCT, COLSG], BF16, tag="lnsq", name="sq")
                    nc.scalar.activation(out=sq[:, :, :], in_=Vi[:, :, :],
                                         func=AF.Square)
                    vib = mam.tile([128, NCT, COLSG], BF16, tag="vib", name="vib")
                    nc.scalar.activation(out=vib[:, :, :], in_=Vi[:, :, :],
                                         func=AF.Copy)
                    s1p = psC.tile([128, COLSG], F32, tag="psS", name=f"s1p{g}")
                    s2p = psC.tile([128, COLSG], F32, tag="psS", name=f"s2p{g}")
                    for ci in range(NCT):
                        nc.tensor.matmul(
                            s1p[0:1, :], ones_col, Vi[:, ci, :],
                            start=(ci == 0), stop=(ci == NCT - 1),
                        )
                        nc.tensor.matmul(
                            s2p[0:1, :], ones_col_bf, sq[:, ci, :],
                            start=(ci == 0), stop=(ci == NCT - 1),
                        )

                    def ln_stats(s1, s2, epsv, tag):
                        m_sb = mam.tile([1, COLSG], F32, tag=f"m{tag}", name="m")
                        nc.vector.tensor_scalar_mul(m_sb, s1[0:1, :], 1.0 / C)
                        v_sb = mam.tile([1, COLSG], F32, tag=f"v{tag}", name="v")
                        nc.vector.tensor_scalar_mul(v_sb, s2[0:1, :], 1.0 / C)
                        msq = mam.tile([1, COLSG], F32, tag=f"msq{tag}", name="msq")
                        nc.vector.tensor_mul(msq, m_sb, m_sb)
                        nc.vector.tensor_sub(v_sb, v_sb, msq)
                        r_sb = mam.tile([1, COLSG], F32, tag=f"r{tag}", name="r")
                        nc.scalar.activation(out=r_sb, in_=v_sb, func=AF.Ln,
                                             bias=epsv)
                        nc.scalar.activation(out=r_sb, in_=r_sb, func=AF.Exp,
                                             scale=-0.5)
                        mr_sb = mam.tile([1, COLSG], F32, tag=f"mr{tag}", name="mr")
                        nc.vector.tensor_mul(mr_sb, m_sb, r_sb)
                        rbm = psC.tile([128, 2, COLSG], F32, tag="psS",
                                       name=f"rbm{tag}{g}")
                        nc.tensor.matmul(rbm[:, 0, :], ones_row, r_sb,
                                         start=True, stop=True)
                        nc.tensor.matmul(rbm[:, 1, :], ones_row, mr_sb,
                                         start=True, stop=True)
                        return rbm[:, 0, :], rbm[:, 1, :]

                    rb1, mrb1 = ln_stats(s1p, s2p, eps1, "a")

                    # ---- P = W_in' @ Vi ; x1 = rb*P + mrb*nwg + wb
                    Pp = psC.tile([128, DIT, COLSG], F32, tag="psS", name=f"Pp{g}")
                    for i in range(DIT):
                        for ci in range(NCT):
                            nc.tensor.matmul(
                                Pp[:, i, :], winT[:, ci, i * 128:(i + 1) * 128],
                                vib[:, ci, :],
                                start=(ci == 0), stop=(ci == NCT - 1),
                            )
                    x1 = mam.tile([128, DIT, COLSG], F32, tag="x1", name=f"x1{g}")
                    rb1_bc = rb1.unsqueeze(1).broadcast_to([128, DIT, COLSG])
                    nc.vector.tensor_copy(out=x1[:, :, :], in_=Pp[:, :, :])
                    nc.vector.tensor_tensor(out=x1[:, :, :], in0=x1[:, :, :],
                                            in1=rb1_bc, op=ALU.mult)
                    for i in range(DIT):
                        nc.vector.scalar_tensor_tensor(
                            out=x1[:, i, :], in0=mrb1[:, :], scalar=nwg[:, i:i + 1],
                            in1=x1[:, i, :], op0=ALU.mult, op1=ALU.add,
                        )
                        nc.vector.tensor_scalar_add(
                            out=x1[:, i, :], in0=x1[:, i, :],
                            scalar1=wb2[:, i:i + 1],
                        )

                    # ---- depthwise conv (along l, guarded per j) + silu
                    cv = mam.tile([128, DIT, COLSG], F32, tag="cv", name=f"cv{g}")
                    e_t = mam.tile([128, DIT, COLSG], F32, tag="e_t", name=f"e_t{g}")
                    xact = mam.tile([128, DIT, COLSG], F32, tag="xact",
                                    name=f"xact{g}")
                    for i in range(DIT):
                        nc.vector.tensor_scalar_mul(
                            out=cv[:, i, :], in0=x1[:, i, :], scalar1=cw3[:, i, 1:2]
                        )
                        cvj = cv[:, i, :].rearrange("p (j l) -> p j l", j=GB)
                        x1j = x1[:, i, :].rearrange("p (j l) -> p j l", j=GB)
                        nc.vector.scalar_tensor_tensor(
                            out=cvj[:, :, 1:L], in0=x1j[:, :, 0:L - 1],
                            scalar=cw3[:, i, 0:1], in1=cvj[:, :, 1:L],
                            op0=ALU.mult, op1=ALU.add,
                        )
                        nc.vector.scalar_tensor_tensor(
                            out=cvj[:, :, 0:L - 1], in0=x1j[:, :, 1:L],
                            scalar=cw3[:, i, 2:3], in1=cvj[:, :, 0:L - 1],
                            op0=ALU.mult, op1=ALU.add,
                        )
                        # silu(cv + cb) = (cv+cb)/(1+exp(-(cv+cb))); ncb = -conv_b
                        nc.scalar.activation(
                            out=e_t[:, i, :], in_=cv[:, i, :], func=AF.Exp,
                            scale=-1.0, bias=ncb[:, i:i + 1],
                        )
                    nc.vector.tensor_scalar_add(out=e_t[:, :, :], in0=e_t[:, :, :],
                                                scalar1=1.0)
                    nc.vector.reciprocal(out=e_t[:, :, :], in_=e_t[:, :, :])
                    for i in range(DIT):
                        nc.vector.scalar_tensor_tensor(
                            out=xact[:, i, :], in0=cv[:, i, :],
                            scalar=ncb[:, i:i + 1],
                            in1=e_t[:, i, :], op0=ALU.subtract, op1=ALU.mult,
                        )

                    # ---- dbc = x @ Wx.T -> [col, 48]; transpose -> [48, col]
                    dbcp = psC.tile([128, 48], F32, tag="psS", name=f"dbcp{g}")
                    for i in range(DIT):
                        nc.tensor.matmul(
                            dbcp[0:COLSG, :], xact[:, i, :], wxT[:, i, :],
                            start=(i == 0), stop=(i == DIT - 1),
                        )
                    dbc_sb = mam.tile([COLSG, 48], F32, tag="dbc", name=f"dbc{g}")
                    nc.vector.tensor_copy(out=dbc_sb, in_=dbcp[0:COLSG, :])
                    dtp = psC.tile([128, COLSG], F32, tag="psS", name=f"dtp{g}")
                    nc.tensor.transpose(
                        dtp[0:48, :], dbc_sb[:, :], ident[0:COLSG, 0:COLSG]
                    )
                    dbcT = mam.tile([48, COLSG], F32, tag="dbcT", name=f"dbcT{g}")
                    nc.vector.tensor_copy(out=dbcT, in_=dtp[0:48, :])

                    # ---- delta = softplus(dT @ Wdt.T + bdt) -> [256, col]
                    delta = mam.tile([128, DIT, COLSG], F32, tag="delta",
                                     name=f"delta{g}")
                    for i in range(DIT):
                        dp = psC.tile([128, COLSG], F32, tag="psS", name=f"dp{g}_{i}")
                        nc.tensor.matmul(
                            dp, wdtT[:, i * 128:(i + 1) * 128], dbcT[0:DTR, :],
                            start=True, stop=True,
                        )
                        nc.scalar.activation(
                            out=delta[:, i, :], in_=dp, func=AF.Exp,
                            bias=bdt2[:, i:i + 1],
                        )
                    nc.scalar.activation(out=delta[:, :, :], in_=delta[:, :, :],
                                         func=AF.Ln, bias=1.0)

                    # ---- Bp/Cp row-broadcast: flat [1, (n j l)] -> [128, (n j l)]
                    bcT_bf = mam.tile([48, COLSG], BF16, tag="bcTb",
                                      name=f"bcTb{g}")
                    nc.vector.tensor_copy(out=bcT_bf, in_=dtp[0:48, :])
                    bc_flat = mam.tile([1, 2 * DS * COLSG], BF16, tag="flat",
                                       name=f"bcf{g}")
                    nc.gpsimd.dma_start(
                        out=bc_flat.rearrange("o (n c) -> o n c", n=2 * DS),
                        in_=bcT_bf[DTR:DTR + 2 * DS, :],
                    )
                    NBC = DS * COLSG
                    bpb = psB.tile([128, NBC], F32, tag="big", bufs=1, name=f"bpb{g}")
                    cpb_ps = psB.tile([128, NBC], F32, tag="big", bufs=1,
                                      name=f"cpp{g}")
                    for k in range((NBC + 511) // 512):
                        sl = slice(512 * k, min(NBC, 512 * (k + 1)))
                        nc.tensor.matmul(bpb[:, sl], ones_row_bf, bc_flat[:, sl],
                                         start=True, stop=True)
                        nc.tensor.matmul(
                            cpb_ps[:, sl], ones_row_bf,
                            bc_flat[:, NBC + 512 * k:NBC + sl.stop],
                            start=True, stop=True)
                    cpb = mam.tile([128, DS * COLSG], BF16, tag="cpbs", name=f"cpb{g}")
                    nc.vector.tensor_copy(out=cpb, in_=cpb_ps)

                    # ---- dA = exp(delta x A), memory order [128, (i n j l)]
                    dA = mam.tile([128, DIT, DS * COLSG], BF16, tag="dA",
                                  name=f"dA{g}")
                    for i in range(DIT):
                        nc.vector.tensor_tensor(
                            out=dA[:, i, :].rearrange("p (n c) -> p n c", n=DS),
                            in0=delta[:, i, :].unsqueeze(1)
                            .broadcast_to([128, DS, COLSG]),
                            in1=A3[:, i, :].unsqueeze(2)
                            .broadcast_to([128, DS, COLSG]),
                            op=ALU.mult,
                        )
                    nc.scalar.activation(out=dA[:, :, :], in_=dA[:, :, :],
                                         func=AF.Exp)
                    # zero dA at l==0 of every (i,n,j) block -> scan resets there
                    nc.vector.tensor_scalar_mul(
                        out=dA.rearrange("p i (b l) -> p (i b) l", l=L)[:, :, 0:1],
                        in0=dA.rearrange("p i (b l) -> p (i b) l", l=L)[:, :, 0:1],
                        scalar1=0.0,
                    )

                    # ---- dBu = (delta*x) x Bp
                    du = mam.tile([128, DIT, COLSG], F32, tag="du", name=f"du{g}")
                    nc.vector.tensor_mul(du[:, :, :], delta[:, :, :], xact[:, :, :])
                    dBu = mam.tile([128, DIT, DS * COLSG], BF16, tag="dBu",
                                   name=f"dBu{g}")
                    for i in range(DIT):
                        nc.vector.tensor_tensor(
                            out=dBu[:, i, :].rearrange("p (n c) -> p n c", n=DS),
                            in0=du[:, i, :].unsqueeze(1)
                            .broadcast_to([128, DS, COLSG]),
                            in1=bpb.rearrange("p (n c) -> p n c", n=DS),
                            op=ALU.mult,
                        )

                    # ---- selective scan: one pass over (i n j l), l innermost
                    Hs = mam.tile([128, DIT * DS * COLSG], BF16, tag="Hs",
                                  name=f"Hs{g}")
                    nc.vector.tensor_tensor_scan(
                        out=Hs[:, :],
                        data0=dA.rearrange("p i c -> p (i c)"),
                        data1=dBu.rearrange("p i c -> p (i c)"),
                        initial=0.0, op0=ALU.mult, op1=ALU.add,
                    )

                    # ---- y = sum_n H * Cp  (+ x * D_ssm)
                    tt = mam.tile([128, DIT, DS * COLSG], BF16, tag="dA",
                                  name=f"yt{g}")
                    nc.vector.tensor_tensor(
                        out=tt[:, :, :],
                        in0=Hs.rearrange("p (i c) -> p i c", i=DIT),
                        in1=cpb.unsqueeze(1).broadcast_to([128, DIT, DS * COLSG]),
                        op=ALU.mult,
                    )
                    y2 = mam.tile([128, DIT, COLSG], F32, tag="y2", name=f"y2{g}")
                    nc.vector.reduce_sum(
                        out=y2[:, :, :],
                        in_=tt.rearrange("p i (n c) -> p i c n", n=DS),
                        axis=AX.X,
                    )
                    for i in range(DIT):
                        nc.vector.scalar_tensor_tensor(
                            out=y2[:, i, :], in0=xact[:, i, :],
                            scalar=dssm2[:, i:i + 1],
                            in1=y2[:, i, :], op0=ALU.mult, op1=ALU.add,
                        )
                    y2b = mam.tile([128, DIT, COLSG], BF16, tag="y2b", name=f"y2b{g}")
                    nc.scalar.activation(out=y2b[:, :, :], in_=y2[:, :, :],
                                         func=AF.Copy)

                    # ---- vi2 = y2 @ W_out.T + Vi/48
                    vi2 = vip.tile([128, NCT, COLSG], F32, tag="v24", bufs=2,
                                   name=f"vi2{g}")
                    vi2p = psB.tile([128, NCT, 64], F32, tag="big", bufs=1,
                                    name=f"vi2p{g}")
                    for mc in range(NCT):
                        for i in range(DIT):
                            nc.tensor.matmul(
                                vi2p[:, mc, 0:COLSG],
                                woutT[:, i, mc * 128:(mc + 1) * 128], y2b[:, i, :],
                                start=(i == 0), stop=(i == DIT - 1),
                            )
                    nc.vector.scalar_tensor_tensor(
                        out=vi2[:, :, :], in0=Vi[:, :, :], scalar=1.0 / POOL_W,
                        in1=vi2p[:, :, 0:COLSG], op0=ALU.mult, op1=ALU.add,
                    )

                    # ---- LN2 (generic g/b)
                    sq2 = mam.tile([128, NCT, COLSG], BF16, tag="lnsq", name="sq2")
                    nc.scalar.activation(out=sq2[:, :, :], in_=vi2[:, :, :],
                                         func=AF.Square)
                    t1p = psC.tile([128, COLSG], F32, tag="psS", name=f"t1p{g}")
                    t2p = psC.tile([128, COLSG], F32, tag="psS", name=f"t2p{g}")
                    for ci in range(NCT):
                        nc.tensor.matmul(
                            t1p[0:1, :], ones_col, vi2[:, ci, :],
                            start=(ci == 0), stop=(ci == NCT - 1),
                        )
                        nc.tensor.matmul(
                            t2p[0:1, :], ones_col_bf, sq2[:, ci, :],
                            start=(ci == 0), stop=(ci == NCT - 1),
                        )
                    rb2, mrb2 = ln_stats(t1p, t2p, eps2, "b")
                    nrm = vip.tile([128, NCT, COLSG], F32, tag="v24", bufs=2,
                                   name=f"nrm{g}")
                    rb2_bc = rb2.unsqueeze(1).broadcast_to([128, NCT, COLSG])
                    mrb2_bc = mrb2.unsqueeze(1).broadcast_to([128, NCT, COLSG])
                    g_bc = ln2g.unsqueeze(2).broadcast_to([128, NCT, COLSG])
                    b_bc = ln2b.unsqueeze(2).broadcast_to([128, NCT, COLSG])
                    nc.vector.tensor_tensor(out=nrm[:, :, :], in0=vi2[:, :, :],
                                            in1=rb2_bc, op=ALU.mult)
                    nc.vector.tensor_tensor(out=nrm[:, :, :], in0=nrm[:, :, :],
                                            in1=mrb2_bc, op=ALU.subtract)
                    nc.vector.tensor_tensor(out=nrm[:, :, :], in0=nrm[:, :, :],
                                            in1=g_bc, op=ALU.mult)
                    nc.vector.tensor_tensor(out=nrm[:, :, :], in0=nrm[:, :, :],
                                            in1=b_bc, op=ALU.add)

                    # ---- channel attention, both streams + mean/max merged.
                    # mvx cols per c: (src(2), s(2), j(2)); col = j*L + 2*pp + s
                    mvx = mam.tile([128, NCT, 4 * GB], F32, tag="mvx",
                                   name=f"mvx{g}")
                    mvx5 = mvx.rearrange("p c (x s j) -> p c x s j", x=2, s=2)
                    nv = nrm.rearrange("p c (j pp two) -> p c two j pp", j=GB,
                                       two=2)
                    for s in range(2):
                        nc.vector.reduce_sum(out=mvx5[:, :, 0, s, :],
                                             in_=nv[:, :, s, :, :], axis=AX.X)
                        nc.vector.reduce_max(out=mvx5[:, :, 1, s, :],
                                             in_=nv[:, :, s, :, :], axis=AX.X)
                    # mean path: sum/6
                    nc.vector.tensor_scalar_mul(
                        out=mvx.rearrange("p c (x sj) -> p c x sj", x=2)[:, :, 0, :],
                        in0=mvx.rearrange("p c (x sj) -> p c x sj", x=2)[:, :, 0, :],
                        scalar1=1.0 / (L // 2),
                    )
                    mvxb = mam.tile([128, NCT, 4 * GB], BF16, tag="mvxb",
                                    name=f"mvxb{g}")
                    nc.scalar.activation(out=mvxb[:, :, :],
                                         in_=mvx[:, :, :], func=AF.Copy)
                    hp = psC.tile([128, COLSG], F32, tag="psS", name=f"hp{g}")
                    for ci in range(NCT):
                        nc.tensor.matmul(
                            hp[:, 0:4 * GB], aw1T[:, ci, :],
                            mvxb[:, ci, :],
                            start=(ci == 0), stop=(ci == NCT - 1),
                        )
                    h1 = mam.tile([128, 2, 2 * GB], BF16, tag="h1", name=f"h1{g}")
                    nc.scalar.activation(
                        out=h1[:, :, :], in_=hp[:, 0:4 * GB], func=AF.Relu,
                        scale=absx, bias=abnb,
                    )
                    h1s = mam.tile([128, 2 * GB], BF16, tag="h1s", name=f"h1s{g}")
                    nc.vector.tensor_tensor(out=h1s, in0=h1[:, 0, :],
                                            in1=h1[:, 1, :], op=ALU.add)
                    att = att_g[g]
                    apb = psB.tile([128, NCT, 2 * GB], F32, tag="big", bufs=1,
                                   name=f"apb{g}")
                    for mc in range(NCT):
                        nc.tensor.matmul(
                            apb[:, mc, :], aw2T[:, mc * 128:(mc + 1) * 128], h1s,
                            start=True, stop=True,
                        )
                    nc.scalar.activation(
                        out=att[:, :, :, :], in_=apb[:, :, :], func=AF.Exp,
                        scale=-1.0,
                    )
                    nc.vector.tensor_scalar_add(out=att[:, :, :, :],
                                                in0=att[:, :, :, :], scalar1=1.0)
                    nc.vector.reciprocal(out=att[:, :, :, :], in_=att[:, :, :, :])

                # ============================================================
                # FFN: per (group, stream) pair of batches, 576-col matmuls.
                # ============================================================
                scl_s = [fvs, fis]
                scl_b = [fvb, fib]
                w_dram = [d_wvT, d_wiT]
                wts = {}

                def load_w(s, eng=None):
                    eng = eng or nc.gpsimd
                    wt = [
                        wpool.tile([128, C], BF16, tag="w", bufs=17,
                                   name=f"w{s}_{kc}")
                        for kc in range(NCT)
                    ]
                    for kc in range(NCT):
                        eng.dma_start(out=wt[kc], in_=w_dram[s][:, kc, :])
                    wts[s] = wt

                def ffn_pair(g, s, skip_io=False):
                    wt = wts[s]
                    att = att_g[g]
                    for j in range(GB):
                        b = g * GB + j
                        axt = [
                            axpool.tile([128, 8, HW], BF16, tag="ax", bufs=4,
                                        name=f"ax{s}_{g}_{j}_{cq}")
                            for cq in range(2)
                        ]
                        if skip_io:
                            for cq in range(2):
                                nc.gpsimd.memset(axt[cq], 0.01)
                        else:
                            for cq in range(2):
                                ft = stream.tile(
                                    [128, 8, HW], F32, tag="fm",
                                    name=f"ffm{s}_{g}_{j}_{cq}",
                                )
                                nc.scalar.dma_start(
                                    out=ft,
                                    in_=fm_d[s][b, cq * 1024:(cq + 1) * 1024, :]
                                    .rearrange("(a p f) w -> p a f w", a=2, f=4),
                                )
                                if cq == 0:
                                    nc.vector.tensor_tensor(
                                        out=axt[cq][:, :, :],
                                        in0=ft[:, :, :],
                                        in1=att[:, 0:8, s, j]
                                        .unsqueeze(2).broadcast_to([128, 8, HW]),
                                        op=ALU.mult,
                                    )
                                else:
                                    for kl in range(8):
                                        nc.scalar.activation(
                                            out=axt[cq][:, kl, :],
                                            in_=ft[:, kl, :], func=AF.Copy,
                                            scale=att[:, 8 + kl, s, j:j + 1],
                                        )
                        for mq in range(NCT // 4):
                            ot = outp.tile(
                                [128, 4, HW], F32, tag="ot",
                                name=f"ot{s}_{g}_{j}_{mq}"
                            )
                            for mi in range(4):
                                mc = mq * 4 + mi
                                pp = psA.tile([128, HW], F32, tag="pp", bufs=4,
                                              name=f"pp{s}_{g}_{j}_{mc}")
                                for kc in range(NCT):
                                    nc.tensor.matmul(
                                        pp, wt[kc][:, mc * 128:(mc + 1) * 128],
                                        axt[kc // 8][:, kc % 8, :],
                                        start=(kc == 0), stop=(kc == NCT - 1),
                                    )
                                nc.scalar.activation(
                                    out=ot[:, mi, :], in_=pp, func=AF.Relu,
                                    scale=scl_s[s][:, mc:mc + 1],
                                    bias=scl_b[s][:, mc:mc + 1],
                                )
                            if not skip_io:
                                nc.scalar.dma_start(
                                    out=out_d[s][b, mq * 512:(mq + 1) * 512, :]
                                    .rearrange("(p four) w -> p four w", four=4),
                                    in_=ot,
                                )

                # ---- emission: software-pipelined fronts and FFN stages
                if parts != "all":
                    load_consts2(nc.gpsimd)
                if parts == "front":
                    for g in range(NG):
                        Vi_g = pool_group(g)
                        mamba_group(g, Vi_g)
                elif parts == "pool":
                    for g in range(NG):
                        pool_group(g)
                elif parts == "mamba":
                    for g in range(NG):
                        Vi_g = vip.tile([128, NCT, COLSG], F32, tag="vi", bufs=NG,
                                        name=f"Vi{g}")
                        nc.vector.memset(Vi_g, 0.5)
                        mamba_group(g, Vi_g)
                elif parts in ("ffn", "mm"):
                    for gg in range(NG):
                        nc.vector.memset(att_g[gg], 1.0)
                    load_w(0)
                    for g in range(NG):
                        ffn_pair(g, 0, skip_io=(parts == "mm"))
                    load_w(1)
                    for g in range(NG):
                        ffn_pair(g, 1, skip_io=(parts == "mm"))
                elif parts == "io":
                    iot = consts.tile([128, 8, HW], F32)
                    nc.vector.memset(iot, 0.25)
                    for s in range(2):
                        for b in range(BL):
                            for cq in range(2):
                                ft = stream.tile([128, 8, HW], F32, tag="fm",
                                                 name=f"ioi{s}_{b}_{cq}")
                                nc.sync.dma_start(
                                    out=ft,
                                    in_=fm_d[s][b, cq * 1024:(cq + 1) * 1024, :]
                                    .rearrange("(a p f) w -> p a f w", a=2, f=4),
                                )
                                nc.scalar.dma_start(
                                    out=out_d[s][b, cq * 1024:(cq + 1) * 1024, :]
                                    .rearrange("(a p f) w -> p a f w", a=2, f=4),
                                    in_=iot,
                                )
                else:
                    Vi0 = pool_group(0)
                    load_consts2(nc.sync)
                    mamba_group(0, Vi0)
                    load_w(0, eng=nc.sync)
                    for g in range(NG):
                        ffn_pair(g, 0)
                        if g + 1 < NG:
                            Vi_g = pool_group(g + 1)
                            mamba_group(g + 1, Vi_g)
                        if g == 0:
                            load_w(1)
                    for g in range(NG):
                        ffn_pair(g, 1)

    nc.compile()
    return nc


# channel permutation: K-tile kc, partition p holds channel (kc//4)*512 + 4*p + (kc%4)
# so each DMA descriptor covers 4 consecutive channels (4.6KB contiguous).
_PERM = np.array(
    [[(kc // 4) * 512 + 4 * p + (kc % 4) for p in range(128)] for kc in range(NCT)]
).reshape(-1)  # [2048] in (kc, p) order


def _host_prep(inputs):
    """Host-side weight layout prep. Returns dict of per-core-replicated arrays."""
    f32 = np.float32
    g = lambda k: np.asarray(inputs[k], dtype=f32)
    s_bn = f32(1.0 / np.sqrt(1.0 + EPS))

    def ctile(v):  # [C] -> [128, 16], channel-permuted
        return np.ascontiguousarray(v[_PERM].reshape(NCT, 128).T)

    def dtile(v):  # [DI] -> [128, 2]
        return np.ascontiguousarray(v.reshape(DIT, 128).T)

    A = -np.exp(g("A_log"))  # [256, 16]
    W_in = g("W_in")
    Wf = W_in * g("ln1_g")[None, :]           # fold ln1 gain into W_in columns
    nwg_v = -(W_in @ g("ln1_g"))              # [256]
    wb_v = W_in @ g("ln1_b")                  # [256]
    sm_parts = {
        "wx": g("Wx").T.reshape(DIT, 128, 48).transpose(1, 0, 2).reshape(128, -1),
        "cw": g("conv_w")[:, 0, :].reshape(DIT, 128, 3).transpose(1, 0, 2).reshape(128, -1),
        "ncb": dtile(-g("conv_b")),
        "bdt": dtile(g("bdt")),
        "dssm": dtile(g("D_ssm")),
        "A3": A.reshape(DIT, 128, DS).transpose(1, 0, 2).reshape(128, -1),
        "nwg": dtile(nwg_v),
        "wb": dtile(wb_v),
        "ln2g": ctile(g("ln2_g")), "ln2b": ctile(g("ln2_b")),
        "absx": (g("att_bn_g") * s_bn)[:, None],
        "abnb": g("att_bn_b")[:, None],
        "fvs": ctile(g("ffn_vis_bn_g") * s_bn),
        "fvb": ctile(g("ffn_vis_b") * (g("ffn_vis_bn_g") * s_bn) + g("ffn_vis_bn_b")),
        "fis": ctile(g("ffn_inf_bn_g") * s_bn),
        "fib": ctile(g("ffn_inf_b") * (g("ffn_inf_bn_g") * s_bn) + g("ffn_inf_bn_b")),
    }
    smalls = np.zeros((128, SM_COLS), f32)
    for name, _w in SMALLS:
        a, b = SM_OFF[name]
        smalls[:, a:b] = sm_parts[name]

    prep = {
        "smalls": smalls,
        "w_inT": np.ascontiguousarray(
            Wf.T[_PERM].reshape(NCT, 128, DI).transpose(1, 0, 2)
        ).astype(ml_dtypes.bfloat16),
        "wdtT": np.ascontiguousarray(g("Wdt").T),
        "w_outT": np.ascontiguousarray(
            g("W_out").T[:, _PERM].reshape(DIT, 128, C).transpose(1, 0, 2)
        ).astype(ml_dtypes.bfloat16),
        "aw1T": np.ascontiguousarray(
            g("att_w1").T[_PERM].reshape(NCT, 128, 128).transpose(1, 0, 2)
        ).astype(ml_dtypes.bfloat16),
        "aw2T": np.ascontiguousarray(g("att_w2").T[:, _PERM]).astype(
            ml_dtypes.bfloat16
        ),
        "wvT": np.ascontiguousarray(
            g("ffn_vis_w").T[_PERM][:, _PERM].reshape(NCT, 128, C).transpose(1, 0, 2)
        ).astype(ml_dtypes.bfloat16),
        "wiT": np.ascontiguousarray(
            g("ffn_inf_w").T[_PERM][:, _PERM].reshape(NCT, 128, C).transpose(1, 0, 2)
        ).astype(ml_dtypes.bfloat16),
    }
    return prep


def _get_runner():
    """Build the bass program once and wrap it in a reusable jitted callable."""
    if "runner" in _CACHE:
        return _CACHE["runner"]

    import jax
    import numpy as _np
    from jax.sharding import Mesh, PartitionSpec
    from jax.experimental.shard_map import shard_map
    import concourse.bacc as bacc
    import concourse.tile as tile
    from concourse import mybir, masks
    from concourse import bass2jax

    nc = _build(bacc, tile, mybir, masks)
    bass2jax.install_neuronx_cc_hook()

    pname = nc.partition_id_tensor.name if nc.partition_id_tensor else None
    in_names, out_names, out_avals, zero_shapes = [], [], [], []
    for alloc in nc.m.functions[0].allocations:
        if not isinstance(alloc, mybir.MemoryLocationSet):
            continue
        name = alloc.memorylocations[0].name
        if alloc.kind == "ExternalInput":
            if name != pname:
                in_names.append(name)
        elif alloc.kind == "ExternalOutput":
            out_names.append(name)
            shape = tuple(alloc.tensor_shape)
            dtype = mybir.dt.np(alloc.dtype)
            out_avals.append(jax.core.ShapedArray(shape, dtype))
            zero_shapes.append((shape, dtype))
    n_params = len(in_names)
    all_names = list(in_names) + list(out_names)
    if pname is not None:
        all_names.append(pname)

    def _body(*args):
        operands = list(args)
        if pname is not None:
            operands.append(bass2jax.partition_id_tensor())
        outs = bass2jax._bass_exec_p.bind(
            *operands,
            out_avals=tuple(out_avals),
            in_names=tuple(all_names),
            out_names=tuple(out_names),
            lowering_input_output_aliases=(),
            sim_require_finite=False,
            sim_require_nnan=False,
            nc=nc,
        )
        return tuple(outs)

    devices = jax.devices()[:N_CORES]
    mesh = Mesh(_np.asarray(devices), ("core",))
    specs = (PartitionSpec("core"),) * (n_params + len(out_names))
    fn = jax.jit(
        shard_map(
            _body,
            mesh=mesh,
            in_specs=specs,
            out_specs=(PartitionSpec("core"),) * len(out_names),
            check_rep=False,
        ),
        keep_unused=True,
    )
    runner = {
        "fn": fn,
        "in_names": in_names,
        "out_names": out_names,
        "zero_shapes": zero_shapes,
        "nc": nc,
    }
    _CACHE["runner"] = runner
    return runner


def kernel(**inputs):
    runner = _get_runner()
    prep = _host_prep(inputs)
    vis = np.asarray(inputs["vis_feat_map"], dtype=np.float32).reshape(B_FULL, C, HW)
    inf = np.asarray(inputs["inf_feat_map"], dtype=np.float32).reshape(B_FULL, C, HW)

    # global inputs: concat of per-core shards along axis 0
    per_in = {"vis": vis, "inf": inf}  # already [64, ...] = 8 cores x [8, ...]
    gin = []
    for name in runner["in_names"]:
        if name in per_in:
            gin.append(per_in[name])
        else:
            arr = prep[name]
            gin.append(np.broadcast_to(arr, (N_CORES,) + arr.shape).reshape(
                (N_CORES * arr.shape[0],) + arr.shape[1:]
            ))
    zeros = [
        np.zeros((N_CORES * s[0],) + tuple(s[1:]), dt)
        for (s, dt) in runner["zero_shapes"]
    ]
    outs = runner["fn"](*gin, *zeros)
    res = {
        name: np.asarray(outs[i]) for i, name in enumerate(runner["out_names"])
    }
    out_vis = res["out_vis"].reshape(B_FULL, C, H, W)
    out_inf = res["out_inf"].reshape(B_FULL, C, H, W)
    return (out_vis, out_inf)
